# revision 49
# baseline (speedup 1.0000x reference)
"""LocallyConnected1D Trainium2 kernel (8-core SPMD, Bass/Tile).

out[b,o,l] = sum_{i,k} x[b,i,l+k] * w[l,o,i,k] + bias[o,l]
  B=64, I=O=128, K=8, L_in=512, L_out=505 (stride 1), fp32 I/O.

Sharding: OUT_LEN across 8 cores (64 positions each, padded 505->512).
Each position is an independent GEMM: out[:, :, l] = X_l @ W_l with
contract dim I*K=1024 split into 8 accumulating 128-contract matmuls.
Weight slice [i, o] is the stationary operand (full 128x128 array),
x window [i, b] streams.

Precision: weights and x are cast to fp8 e3m4 on host (the weight DMA
is the roofline: 265MB fp32 -> 66MB fp8), PSUM accumulates fp32, bias
is added in fp32 on DVE, and the output is written back bf16 and
upcast to fp32 on host. Measured end-to-end rel err 1.68e-2 (L2) /
1.77e-2 (max, absmax-scaled) on the fixed-seed reference inputs —
under the 2e-2 gate; set x_fp8=False (bf16 x, 33.8us) for 1.20e-2.

Schedule (tuned against TimelineSim, HW-verified 30491ns = 1300 head
+ ~27921 stream + ~137 tail gaps + 900 DMA-sem epilogue + 233 exit
barrier). The stream runs at the model's full 360GB/s with zero
mid-stream gaps, so everything after the fp8 cast is tail/head work:
- weight blocks taper at both ends ((2,2,4)+(8,)*5+(4,4,2,2,2,1,1)):
  small head blocks start the PE early; the fine late taper keeps
  PE's last-16-position stretch sem-locked to each block's arrival
  instead of queued behind one big 8-block semaphore.
- the last position's weights are split 4+4 taps (split_last_tap=4,
  both 512B/partition descriptors, no sub-512B 2x penalty): after the
  final 46KB morsel's sem only 4 matmuls + one bias-add remain on the
  critical chain.
- bias-adds for positions 56-62 ride the idle Activation engine
  (nc.scalar.add) so DVE is free to run position 63's bias the moment
  its PSUM lands (DVE's 216ns/op tail queue otherwise delays it).
  GPSIMD/Pool cannot read PSUM (BIR verifier rejects it).
- out blocks (48,8,4,4) ride after the last weight bytes: out1/out2 on
  Pool SWDGE, out3+final on SP HWDGE (out3's dispatch fires on
  bias59's sem, well before the final block's chain, so SP's in-order
  sequencer never stalls the final dispatch).
- x arrives in (2+15)+8*7 column chunks (each >=512B/partition). The
  bf16 bias no longer gets its own DMA (128B/partition would pay the
  sub-512B 2x penalty): its raw bytes ride as two leading fp8 columns
  of x inside x0's single 1088B-descriptor transfer, recovered on-chip
  by a flatten+bitcast view and upcast once on Activation (DVE's
  tensor_scalar bias operand must be fp32). Saves a net 45ns of
  stream time and one SWDGE dispatch.
IR post-passes (TimelineSim and the NEFF see the same mutated IR):
the first 3 wait-free weight DMAs are hoisted above the framework
preamble (DMA pipe overlaps the ~1us engine-start rendezvous) and the
trailing exit-barrier instructions after Pool's ISA are dropped
(trim_exit=1). The deeper trim (trim_exit=2, -233ns in sim) removes
the second barrier round entirely and WEDGES the real device
(NRT_EXEC_UNIT_UNRECOVERABLE) — do not enable it.
wb_outs (default ON, HW-verified 29011ns with bias_in_x / rel err
identical to the 30718ns baseline): all
four out blocks leave via prepared SWDGE kv_writebacks instead of
DMACopies. Each block's prep (descriptor gen, ~1us on the idle Pool
engine) is emitted after its bias-adds — so Tile defers the RAW waits
onto the trigger — then relocated early in IR with its waits moved
onto the trigger (the trigger is the actual read point; the prep
itself only needs the idxt memset, which precedes it in Pool program
order — preps MUST land after that memset or the Q7 reads garbage ctx
indices and the OOB guard silently skips every write). Blocks 1-3
fire from one combined trigger emitted just before the final prep
(their transfers ride the idle post-stream window instead of cutting
into the weight stream); the final block's trigger fires ~60ns after
bias63's sem, replacing the 650+650 dispatch+DGE chain. Each kv
"batch" writes a pow2 (<=2048B) contiguous ctx run (out1 = 3x16
positions, ncn=1024), so descriptors stay big. Making this execute on
real HW needs two Bacc passes replayed here: insert_library_loads
(kv_writeback's Q7 ucode lives in the reloadable 'attn' library —
without the MODIFY_POOL_CONFIG load the device faults unrecoverably)
and lower_extended_insts (populates extended-inst .instr bytes; else
walrus fails with "ISA wrong length"). The framework's exit
flush-drain expects the SWDGE queue sems the preps no longer post;
an IR pass clamps those waits to actual posts and waits the custom
wb_out sem instead. Remaining structure is at its floor: 1300 head
+ ~24963 input-byte stream + 900 w-sem + ~420 PE/bias chain + ~75
trigger+transfer + 900 out-sem + ~233 exit barrier. Probed and dead:
prepared-gather head start (SWDGE gen 994ns + prep-done sem lands
first bytes at ~1430 vs HWDGE's 1300), sub-8-bit weights (break the
2e-2 gate), every exit-barrier trim (wedges the device), dma_transpose
weight loads (cost model's 14ns/16x128-tile matches byte rate and the
instruction is 2-byte-dtype-only).

kernel() retries the fast build once after a 25s pause (a transiently
wedged device — e.g. from a prior faulting tenant — usually recovers),
then falls back to a plain-DMA build (30491ns, no extended
instructions) so a degraded device still produces a correct result.
"""

import json

import numpy as np
import ml_dtypes

B = 64
IC = 128
OC = 128
KW = 8
LIN = 512
LOUT = 505
NCORES = 8
LPC = 64  # padded positions per core: 8*64 = 512 >= 505
TW = LPC + KW - 1  # x time-columns a core touches (71)
TPAD = (NCORES - 1) * LPC + TW  # padded x length (519)
OB = 8  # x-chunk width (columns) and w/out block alignment granularity

_BF16 = ml_dtypes.bfloat16
_F8 = ml_dtypes.float8_e3m4

_CACHE: dict = {}
LAST_RESULTS = None  # BassKernelResults of the most recent kernel() call


def _hoist_head_dmas_ir(nc, n: int = 2, top: bool = False) -> None:
    """Move the first `n` wait-free SP DMACopy instructions from the body
    block into the preamble block, after SP's RegisterMoves but before the
    start barrier. The DMA pipe (dispatch+HWDGE+DGE delay) then overlaps the
    ~1us engine-start rendezvous, starting the weight stream ~0.8us earlier.
    Safe because the hoisted DMAs wait on nothing, nothing reads their tiles
    until their completion semaphores fire (well after the preamble), and
    SP's own preamble order (RegisterMoves first) is preserved. Mutates the
    in-memory IR so TimelineSim and the NEFF see the same program."""
    import concourse.mybir as mybir

    blocks = nc.m.functions[0].blocks
    if len(blocks) < 2:
        return
    pre, body = blocks[0].instructions, blocks[1].instructions
    hoist = []
    for inst in body:
        if len(hoist) >= n:
            break
        si = getattr(inst, "sync_info", None)
        waits = getattr(si, "on_wait", None) if si is not None else None
        if (type(inst).__name__ == "InstDMACopy"
                and inst.engine == mybir.EngineType.SP and not waits):
            hoist.append(inst)
    if not hoist:
        return
    ids = {id(i) for i in hoist}
    kept = [i for i in body if id(i) not in ids]
    del body[:]
    body.extend(kept)
    if top:
        idx = 1  # right after the framework dummy Call
    else:
        idx = max(i for i, inst in enumerate(pre)
                  if inst.engine == mybir.EngineType.SP
                  and type(inst).__name__ == "InstRegisterMove") + 1
    for k, inst in enumerate(hoist):
        pre.insert(idx + k, inst)


def _trim_exit_barrier_ir(nc, deep: bool = False) -> None:
    """Drop the second (redundant) all-engine barrier round at program exit.
    Round 1 already rendezvouses after SP's big DMA-flush drain (the W:16
    wait on every DMA-completion semaphore), so outputs are in DRAM before
    any engine passes it; the trailing Pool ISA op is kept as the final
    instruction."""
    blocks = nc.m.functions[0].blocks
    exit_insts = blocks[-1].instructions
    isa_idx = [i for i, inst in enumerate(exit_insts)
               if type(inst).__name__ == "InstISA"]
    if not isa_idx:
        return
    if deep:
        # keep every engine's Drain/ISA teardown instructions (nrt needs
        # each engine's stream to terminate properly) but strip the
        # cross-engine barrier EventSemaphores and their waits so engines
        # finish independently instead of paying the ~233ns second
        # rendezvous round
        kept = []
        for i, inst in enumerate(exit_insts[: isa_idx[-1] + 1]):
            tn = type(inst).__name__
            if tn == "InstEventSemaphore":
                continue
            if tn in ("InstDrain", "InstISA"):
                si = getattr(inst, "sync_info", None)
                if i > 0 and si is not None and getattr(si, "on_wait", None):
                    del si.on_wait[:]
                kept.append(inst)
    else:
        if isa_idx[-1] == len(exit_insts) - 1:
            return
        kept = exit_insts[: isa_idx[-1] + 1]
    del exit_insts[:]
    exit_insts.extend(kept)


# --- workaround: this walrus build rejects >1 sync wait per instruction ----
def _split_waits(raw: bytes) -> bytes:
    m = json.loads(raw)
    ctr = 0
    for f in m.get("functions", []):
        for blk in f.get("blocks", []) or f.get("basicblocks", []):
            out = []
            for inst in blk.get("instructions", []):
                si = inst.get("sync_info")
                waits = (si or {}).get("on_wait") or []
                if len(waits) > 1:
                    for w in waits[:-1]:
                        ctr += 1
                        out.append(
                            {
                                "debug": inst.get("debug", 0),
                                "engine": inst["engine"],
                                "ins": [],
                                "name": f"waitsplit_{ctr}",
                                "opcode": "EventSemaphore",
                                "outs": [],
                                "sync_info": {"on_update": [], "on_wait": [w]},
                            }
                        )
                    si["on_wait"] = waits[-1:]
                out.append(inst)
            blk["instructions"] = out
    return json.dumps(m).encode()


def _build_bass(w_bufs: int = 3, psum_bufs: int = 8, out_bufs: int = 4,
                w_sched=(2, 2, 4) + (8,) * 5 + (4, 4, 2, 2, 2, 1, 1),
                out_sched=(48, 8, 4, 4),
                xa_cols: int = 15, out_eng: str = "gpsimd",
                bias_eng: str = "gpsimd", x0_eng: str = "gpsimd",
                x_eng: str = "sync", final_out_eng: str | None = "sync",
                sync_last_n_outs: int = 1, split_last_tap: int = 4,
                act_bias_last_n: int = 0,
                out_engs=("gpsimd", "gpsimd", "sync", "sync"),
                act_bias_ranges=(),
                bias_engs=((56, 63, "scalar"),),
                split_prefetch: int = 1, bias_in_x: bool = True,
                x_prefetch_all: bool = False, x_fp8: bool = True,
                hoist_head: int = 3, hoist_top: bool = True,
                trim_exit: int = 1, scatter_final: bool = False,
                wb_final: bool = False, wb_outs: bool = True,
                wb_defer_trigger: bool = False,
                reps: int = 1):
    import contextlib

    import concourse.bass as bass
    import concourse.tile as tile
    import concourse.mybir as mybir

    sched = list(w_sched)
    assert sum(sched) == LPC
    osched = list(out_sched)
    assert sum(osched) == LPC
    # w blocks must not straddle out blocks
    obounds = [0]
    for nb in osched:
        obounds.append(obounds[-1] + nb)
    acc = 0
    for nb in sched:
        assert any(a <= acc and acc + nb <= b
                   for a, b in zip(obounds[:-1], obounds[1:]))
        acc += nb

    # x column chunks: [0, xa_cols) then OB-wide chunks to TW
    xbounds = [0, xa_cols]
    while xbounds[-1] < TW:
        xbounds.append(min(xbounds[-1] + OB, TW))

    xdt = mybir.dt.float8e3 if x_fp8 else mybir.dt.bfloat16

    if wb_outs:
        wb_final = True
    nc = bass.Bass(num_swdge_queues=2 if wb_final else 1)
    # bias_in_x: the 128 bias bytes per partition ride as two extra fp8
    # columns at the HEAD of x (cols 0-1), so they move inside x0's single
    # >=512B-descriptor transfer instead of a separate 128B/partition DMA
    # that pays the sub-512B 2x latency penalty (91ns -> +45ns net save)
    xcols = TW + 2 if bias_in_x else TW
    x_d = nc.dram_tensor("x", [IC, xcols, B], xdt, kind="ExternalInput")
    w_d = nc.dram_tensor(
        "w", [IC, LPC, KW, OC], mybir.dt.float8e3, kind="ExternalInput"
    )
    if not bias_in_x:
        b_d = nc.dram_tensor("bias", [OC, LPC], mybir.dt.bfloat16,
                             kind="ExternalInput")
    if scatter_final:
        ix_d = nc.dram_tensor("idx", [128, 8], mybir.dt.int16,
                              kind="ExternalInput")
    o_d = nc.dram_tensor("out", [OC, LPC, B], mybir.dt.bfloat16, kind="ExternalOutput")

    # out DMAs go on their own queue: their compute-dependency waits must not
    # block later weight-block DMAs behind them on SP's in-order sequencer
    oeng = getattr(nc, out_eng)

    with tile.TileContext(nc) as tc:
        with (
            tc.tile_pool(name="const", bufs=1) as constp,
            tc.tile_pool(name="wp", bufs=w_bufs) as wp,
            tc.tile_pool(name="op", bufs=out_bufs) as op,
            tc.tile_pool(name="ps", bufs=psum_bufs, space="PSUM") as pp,
        ):
            # x chunk tiles; chunk 0 lands first so the PE can start early
            xtiles = []  # (start_col, tile)
            nchunks = len(xbounds) - 1
            xoff = 2 if bias_in_x else 0  # bias cols precede x col 0
            for ci in range(nchunks):
                c0, c1 = xbounds[ci], xbounds[ci + 1]
                w_extra = xoff if ci == 0 else 0
                xt = constp.tile([IC, c1 - c0 + w_extra, B], xdt,
                                 name=f"x{ci}", tag=f"x{ci}")
                xtiles.append((c0, xt))
            xdma_done = [False] * nchunks

            def need_x(col):
                ci = next(i for i in range(nchunks)
                          if xbounds[i] <= col < xbounds[i + 1])
                if not xdma_done[ci]:
                    c0, xt = xtiles[ci]
                    d0 = 0 if ci == 0 else c0 + xoff
                    eng = getattr(nc, x0_eng if ci == 0 else x_eng)
                    eng.dma_start(xt[:], x_d[:, d0: d0 + xt.shape[1]])
                    xdma_done[ci] = True
                return ci

            def x_ap(col):
                ci = need_x(col)
                c0, xt = xtiles[ci]
                return xt[:, col - c0 + (xoff if ci == 0 else 0), :]

            need_x(0)
            if bias_in_x:
                # recover the bf16 bias from x0's leading two byte-columns
                bth = (xtiles[0][1][:, 0:2, :]
                       .rearrange("p a b -> p (a b)")
                       .bitcast(mybir.dt.bfloat16))
            else:
                bth = constp.tile([OC, LPC], mybir.dt.bfloat16)
                getattr(nc, bias_eng).dma_start(bth[:], b_d[:])
            fnb = osched[-1]
            ot_final = None
            wb_sem = None
            if wb_final:
                # final out block leaves via a prepared SWDGE kv_writeback:
                # descriptors are generated early on Pool; at the tail a
                # ~60ns trigger_dma (no dispatch/HWDGE/DGE chain) fires the
                # transfer as soon as the last bias-add lands. The whole
                # [OC, fnb, B] block is written as ONE kv "batch" with a
                # (fnb*B)-element contiguous ctx run -> 512B descriptors,
                # dodging the sub-512B 2x penalty.
                def wb_geom(onb):
                    p = 1
                    while p * 2 <= onb and onb % (p * 2) == 0 \
                            and (p * 2) * B <= 2048:
                        p *= 2
                    return onb // p, p  # (kv batch, positions per batch)

                max_bt = max(wb_geom(nb)[0] for nb in osched) if wb_outs else 1
                idxt = constp.tile([128, max_bt], mybir.dt.int32, name="wbidx")
                # memset on Pool: program order guarantees it precedes the
                # (wait-stripped, relocated) preps on the same sequencer
                nc.gpsimd.memset(idxt[:], 0)
                sem_ctx = nc.semaphore("wb_out")
                wb_sem = sem_ctx.__enter__()
                wb_expect = 0

                def wb_view(ol0, onb):
                    bt, p = wb_geom(onb)
                    return (o_d[:, ol0: ol0 + onb, :]
                            .rearrange("(oi oo) (bt li) b -> bt oi oo (li b)",
                                       oo=1, bt=bt))

                fbt, fp = wb_geom(fnb)
                ot_final = op.tile([OC, 1, fbt, fp * B], mybir.dt.bfloat16,
                                   name="otf", tag="otf")
                wb_out_ap = wb_view(LPC - fnb, fnb)
            if scatter_final:
                # final out block goes out via a pre-prepared SWDGE scatter:
                # descriptors are generated early; at the tail only a ~40ns
                # trigger separates the last bias-add from the transfer,
                # replacing the 153+625+650 dispatch+HWDGE+DGE-delay chain.
                # scatter ADDs, so zero the target DRAM region first (early).
                zt = constp.tile([OC, fnb * B], mybir.dt.bfloat16, name="zt")
                nc.vector.memset(zt[:], 0)
                nc.sync.dma_start(o_d[:, LPC - fnb:, :], zt[:])
                idxt = constp.tile([128, 8], mybir.dt.int16, name="idxt")
                nc.sync.dma_start(idxt[:], ix_d[:])
                ot_final = op.tile([OC, 1, fnb * B], mybir.dt.bfloat16,
                                   name="otf", tag="otf")
                import contextlib as _cl
                sem_ctx = nc.semaphore("scat_out")
                scat_sem = sem_ctx.__enter__()
                nc.gpsimd.dma_scatter_add(
                    out_ap=o_d[:, LPC - fnb:, :].opt({0}),
                    in_ap=ot_final[:],
                    idxs_ap=idxt[:],
                    num_idxs=128,
                    num_idxs_reg=128,
                    elem_size=fnb * B,
                    elem_step=LPC * B,
                    prepare_only=True,
                    sem=scat_sem,
                )

            if x_prefetch_all is True:
                # stream order doesn't change when the last w block lands
                # (pool is serial, bytes are bytes), but early x makes every
                # tail-position x-semaphore long-satisfied by drain time
                for ci in range(nchunks):
                    need_x(xbounds[ci])
            # tensor_scalar_add needs an fp32 scalar operand: upcast once on
            # the (otherwise idle) Activation engine, off the critical path
            bt = constp.tile([OC, LPC], mybir.dt.float32)
            nc.scalar.copy(bt[:], bth if bias_in_x else bth[:])

            if reps > 1:  # timing mode: hoist x loads out of the repeat loop
                for ci in range(nchunks):
                    need_x(xbounds[ci])

            blocks = []  # (l0, nb)
            l0 = 0
            for nb in sched:
                blocks.append((l0, nb))
                l0 += nb

            rep_ctx = tc.For_i(0, reps, 1) if reps > 1 else contextlib.nullcontext()
            with rep_ctx:
                bi = 0  # next block to process
                wt = None
                wt_tap = None
                wl0 = wnb = 0
                for ol0, onb in zip(obounds[:-1], osched):
                    is_final = ol0 + onb == LPC
                    wb_block = wb_outs or (wb_final and is_final)
                    if (scatter_final or wb_final) and is_final:
                        ot = ot_final
                        obt, opp = (fbt, fp) if wb_final else (1, onb)
                    elif wb_block:
                        obt, opp = wb_geom(onb)
                        ot = op.tile([OC, 1, obt, opp * B],
                                     mybir.dt.bfloat16,
                                     name=f"ot{onb}", tag=f"ot{onb}")
                    else:
                        ot = op.tile([OC, onb, B], mybir.dt.bfloat16,
                                     name=f"ot{onb}", tag=f"ot{onb}")
                    for j in range(onb):
                        l = ol0 + j
                        if wt is None or l >= wl0 + wnb:
                            wl0, wnb = blocks[bi]
                            bi += 1
                            do_split = (split_last_tap
                                        and blocks[-1][1] == 1)
                            if do_split and bi == len(blocks):
                                # final position: leading taps were prefetched
                                # a block early (below); only the last
                                # split_last_tap taps arrive last, shrinking
                                # the post-arrival critical chain
                                nt = int(split_last_tap)
                                wt = wtf_a
                                wt_tap = wp.tile([IC, 1, nt, OC],
                                                 mybir.dt.float8e3,
                                                 name="wtf_b", tag="wtf_b")
                                nc.sync.dma_start(
                                    wt_tap[:], w_d[:, wl0: wl0 + 1, KW - nt:])
                            else:
                                wt = wp.tile([IC, wnb, KW, OC],
                                             mybir.dt.float8e3,
                                             name=f"wt{wnb}", tag=f"wt{wnb}")
                                wt_tap = None
                                # prefetch x chunks this block touches first
                                need_x(wl0 + wnb - 1 + KW - 1)
                                nc.sync.dma_start(wt[:], w_d[:, wl0: wl0 + wnb])
                            if (isinstance(x_prefetch_all, int)
                                    and x_prefetch_all is not True
                                    and x_prefetch_all > 0
                                    and bi == x_prefetch_all):
                                # deferred full-x prefetch: PE has ramped on
                                # the early blocks; remaining x rides now so
                                # tail x-semaphores are long satisfied
                                for ci in range(nchunks):
                                    need_x(xbounds[ci])
                            if do_split and bi == len(blocks) - split_prefetch:
                                nt = int(split_last_tap)
                                fl0 = blocks[-1][0]
                                wtf_a = wp.tile([IC, 1, KW - nt, OC],
                                                mybir.dt.float8e3,
                                                name="wtf_a", tag="wtf_a")
                                need_x(fl0 + KW - 1)
                                nc.sync.dma_start(
                                    wtf_a[:], w_d[:, fl0: fl0 + 1, : KW - nt])
                        ps = pp.tile([OC, B], mybir.dt.float32)
                        for k in range(KW):
                            if wt_tap is not None and k >= KW - int(split_last_tap):
                                src = wt_tap[:, l - wl0, k - (KW - int(split_last_tap)), :]
                            else:
                                src = wt[:, l - wl0, k, :]
                            nc.tensor.matmul(
                                ps[:],
                                src,
                                x_ap(l + k),
                                start=(k == 0),
                                stop=(k == KW - 1),
                            )
                        if scatter_final and is_final:
                            nc.vector.tensor_scalar_add(
                                ot_final[:, 0, j * B: (j + 1) * B],
                                ps[:], bt[:, l: l + 1]
                            )
                        elif wb_block:
                            jb, jl = divmod(j, opp)
                            tgt = ot[:, 0, jb, jl * B: (jl + 1) * B]
                            weng = next(
                                (e for a, b, e in bias_engs if a <= l < b),
                                "vector")
                            if weng == "scalar":
                                nc.scalar.add(tgt, ps[:], bt[:, l: l + 1])
                            else:
                                getattr(nc, weng).tensor_scalar_add(
                                    tgt, ps[:], bt[:, l: l + 1])
                        elif (beng := next(
                                (e for a, b, e in bias_engs if a <= l < b),
                                "scalar" if (l >= LPC - act_bias_last_n
                                             or any(a <= l < b
                                                    for a, b in act_bias_ranges))
                                else None)) is not None:
                            # tail positions: bias-add off DVE (Activation's
                            # add or Pool's tensor_scalar_add) to dodge DVE's
                            # 216ns/op tail queue
                            if beng == "scalar":
                                nc.scalar.add(ot[:, j, :], ps[:],
                                              bt[:, l: l + 1])
                            else:
                                getattr(nc, beng).tensor_scalar_add(
                                    ot[:, j, :], ps[:], bt[:, l: l + 1])
                        else:
                            nc.vector.tensor_scalar_add(
                                ot[:, j, :], ps[:], bt[:, l: l + 1]
                            )
                    if scatter_final and is_final:
                        nc.gpsimd.trigger_dma(count=None)
                        nc.gpsimd.wait_ge(scat_sem, 1)
                        continue
                    if wb_block:
                        if is_final:
                            # fire blocks 1..3's prepared writebacks now
                            # (one combined trigger, gated by their bias
                            # sems via the preps' deferred deps): their
                            # transfers ride the idle post-stream window
                            # instead of cutting into the weight stream
                            nc.gpsimd.trigger_dma(count=None, queue_num=1)
                        # emitted after the bias-adds so Tile defers the
                        # prep's RAW waits onto the trigger (prep itself is
                        # wait-free and gets hoisted early in IR below)
                        nc.gpsimd.kv_writeback(
                            out_ap=(wb_out_ap if is_final
                                    else wb_view(ol0, onb)),
                            in_ap=ot[:],
                            ctx_idxs_ap=idxt[:, :obt],
                            prepare_only=True,
                            sem=wb_sem,
                            queue_num=1,
                        )
                        wb_expect += 16
                        if is_final:
                            nc.gpsimd.trigger_dma(count=None, queue_num=1)
                        continue
                    oidx = obounds.index(ol0)
                    if out_engs is not None:
                        eng = getattr(nc, out_engs[oidx])
                    else:
                        eng = oeng
                        if final_out_eng is not None and oidx >= len(osched) - sync_last_n_outs:
                            eng = getattr(nc, final_out_eng)
                    eng.dma_start(o_d[:, ol0: ol0 + onb, :], ot[:])

    if scatter_final or wb_final:
        # Tile sinks the prepare next to its trigger at the program tail,
        # putting the ~1us SWDGE descriptor generation on the critical chain
        # (and starving the trigger's no_exec FIFO visit). Move it early: its
        # only wait is the idx tile (memset/DMA, ~2us); parking Pool's
        # sequencer on that is harmless since the next Pool work (out
        # dispatches) is much later.
        prep_ty = ("InstDMAScatterAddAnt" if scatter_final
                   else "InstKVWritebackAnt")
        body = nc.m.functions[0].blocks[1].instructions
        preps = [inst for inst in body if type(inst).__name__ == prep_ty]
        prep = preps[0]
        if wb_final:
            # The prep's data read happens at trigger time, but the emitted
            # sync waits (on the bias-adds that fill its source tile) sit on
            # the prep and would park Pool's sequencer until the tail. MOVE
            # each prep's waits onto its trigger — the trigger is the actual
            # read point, so the data dependency stays sound while the prep
            # (descriptor gen only) runs early. The prep's remaining dep, the
            # idxt memset, precedes it in Pool program order.
            for _p in preps:
                _si = getattr(_p, "sync_info", None)
                if _si is None or not getattr(_si, "on_wait", None):
                    continue
                _trig = None
                _seen = False
                for _inst in body:
                    if _inst is _p:
                        _seen = True
                    elif _seen and type(_inst).__name__ == "InstTriggerDma":
                        _trig = _inst
                        break
                assert _trig is not None, "prep without trigger"
                _tsi = _trig.sync_info
                _have = {(w.id, w.wait_mode): w
                         for w in (_tsi.on_wait or [])}
                for w in _si.on_wait:
                    k = (w.id, w.wait_mode)
                    if k in _have:
                        _have[k].wait_value = max(
                            _have[k].wait_value or 0, w.wait_value or 0)
                    else:
                        _tsi.on_wait.append(w)
                        _have[k] = w
                del _si.on_wait[:]
        if True:
            kept = [inst for inst in body
                    if type(inst).__name__ != prep_ty]
            del body[:]
            body.extend(kept)
        import concourse.mybir as _mb
        n_pool = 0
        ins_at = 0
        for i, inst in enumerate(body):
            if (inst.engine == _mb.EngineType.Pool
                    and type(inst).__name__ == "InstDMACopy"):
                n_pool += 1
                if n_pool == 2:  # after x0 and bias dispatches
                    ins_at = i + 1
                    break
        if wb_final:
            # the preps' descriptor-gen READS the idxt tile at gen time:
            # they must land after its Pool memset (and after the library
            # reload), or the Q7 reads garbage ctx indices and the OOB
            # guard silently skips every write
            for i in range(ins_at, len(body)):
                inst = body[i]
                if (inst.engine == _mb.EngineType.Pool
                        and type(inst).__name__ == "InstMemset"):
                    ins_at = i + 1
                    break
        for k, _p in enumerate(preps):
            body.insert(ins_at + k, _p)
    if wb_final:
        # The framework's exit flush-drain waits every DMA-queue sem to its
        # expected final value, but the prepared writeback posts its 16
        # completion bumps to the custom wb_out sem instead of its SWDGE
        # queue's builtin sem. Clamp each drain wait to what the program
        # actually posts (drop if nothing does) and wait the wb sem
        # explicitly so the drain still covers the writeback's landing.
        posted: dict[int, int] = {}
        wb_id = None
        for blk in nc.m.functions[0].blocks:
            for inst in blk.instructions:
                si = getattr(inst, "sync_info", None)
                for upd in (getattr(si, "on_update", None) or []):
                    if str(getattr(upd, "update_mode", "")) in (
                            "sem-inc", "sem-add-imm"):
                        posted[upd.id] = posted.get(upd.id, 0) + (
                            upd.update_value or 0)
                        if (upd.ant_name or "").startswith("wb_out"):
                            wb_id = upd.id
        import concourse.mybir as _mb2
        for inst in nc.m.functions[0].blocks[-1].instructions:
            si = getattr(inst, "sync_info", None)
            waits = (getattr(si, "on_wait", None) or [])
            if not waits or type(inst).__name__ != "InstDrain":
                continue
            new_waits = []
            patched = False
            for w in waits:
                nm = w.ant_name or ""
                if (nm.startswith("DMASW") or nm.startswith("DMAHW")):
                    have = posted.get(w.id, 0)
                    if have <= 0:
                        patched = True
                        continue  # nothing posts: unsatisfiable, drop
                    if have < (w.wait_value or 0):
                        w.wait_value = have
                        patched = True
                new_waits.append(w)
            if patched and wb_id is not None:
                new_waits.append(_mb2.SyncWait(
                    sync_type="semaphore", id=wb_id, ant_name="wb_out",
                    wait_mode="sem-ge-imm",
                    wait_value=posted.get(wb_id, 16), wait_reg=None))
            if patched:
                del si.on_wait[:]
                si.on_wait.extend(new_waits)
    if hoist_head > 0:
        _hoist_head_dmas_ir(nc, n=hoist_head, top=hoist_top)
    if trim_exit:
        _trim_exit_barrier_ir(nc, deep=(trim_exit == 2))
    if wb_final or scatter_final:
        # kv_writeback's Q7 ucode lives in the reloadable 'attn' library,
        # not the boot default. Bacc.compile runs the insert_library_loads
        # pass to place MODIFY_POOL_CONFIG loads before instructions that
        # need a non-resident library (and back-switches for standard-lib
        # ops); raw Bass never does, so run the same rust pass here.
        from concourse.library_config import all_libraries, standard
        import bass_rust as _br
        _mask: dict = {}
        for _lib in all_libraries:
            for _it in _lib.instructions:
                _mask[_it] = _mask.get(_it, 0) | (1 << _lib.index)
        _br.insert_library_loads(nc, _mask, len(all_libraries),
                                 standard.index)
        # extended insts (kv_writeback / trigger_dma / IncSwdgeSem) carry
        # their ISA encodings in .instr — raw Bass never populates them
        # (Bacc.compile does); without this the NEFF codegen fails with
        # "ISA wrong length"
        from concourse.library_overlay import lower_extended_insts
        lower_extended_insts(nc)
    fixed = _split_waits(bass.Bass.to_json_bytes(nc))
    nc.to_json_bytes = lambda: fixed  # type: ignore[method-assign]
    return nc


def _prepare_inputs(x, weight, bias, x_fp8=True, bias_in_x=True):
    x = np.asarray(x, dtype=np.float32)
    weight = np.asarray(weight, dtype=np.float32)
    bias = np.asarray(bias, dtype=np.float32)

    # x: [b, i, t] -> bf16/fp8, pad t to TPAD, transpose -> [i, t, b]
    xdt = _F8 if x_fp8 else _BF16
    xpad = np.zeros((B, IC, TPAD), dtype=xdt)
    xpad[:, :, :LIN] = x.astype(xdt)
    xt = xpad.transpose(1, 2, 0)  # [i, t, b] view

    # weight: [l, o, i, k] -> fp8 e3m4, pad l, transpose -> [i, l, k, o]
    wpad = np.zeros((NCORES * LPC, OC, IC, KW), dtype=_F8)
    wpad[:LOUT] = weight.astype(_F8)
    wt = wpad.transpose(2, 0, 3, 1)  # [i, l, k, o] view

    bpad = np.zeros((OC, NCORES * LPC), dtype=_BF16)
    bpad[:, :LOUT] = bias.astype(_BF16)

    in_maps = []
    for c in range(NCORES):
        l0 = c * LPC
        if bias_in_x:
            # bias [OC, LPC] bf16 -> 128 raw bytes per partition -> two
            # leading fp8 byte-columns of x (matches the on-chip bitcast:
            # free-dim-contiguous little-endian bf16 pairs)
            bb = (np.ascontiguousarray(bpad[:, l0: l0 + LPC])
                  .view(np.uint8)          # [OC, 2*LPC]
                  .reshape(IC, 2, B)
                  .view(xdt if x_fp8 else np.uint8))
            if not x_fp8:
                raise NotImplementedError("bias_in_x requires x_fp8")
            xc = np.concatenate(
                [bb, np.ascontiguousarray(xt[:, l0: l0 + TW, :])], axis=1)
            in_maps.append(
                {
                    "x": np.ascontiguousarray(xc),
                    "w": np.ascontiguousarray(wt[:, l0: l0 + LPC]),
                }
            )
        else:
            in_maps.append(
                {
                    "x": np.ascontiguousarray(xt[:, l0: l0 + TW, :]),
                    "w": np.ascontiguousarray(wt[:, l0: l0 + LPC]),
                    "bias": np.ascontiguousarray(bpad[:, l0: l0 + LPC]),
                }
            )
    return in_maps


def _assemble(results):
    full = np.stack([results[c]["out"] for c in range(NCORES)], axis=0)
    # [c, o, l_loc, b] (bf16) -> fp32 [b, o, c*LPC + l_loc] -> crop to LOUT
    out = (
        full.astype(np.float32)
        .transpose(3, 1, 0, 2)
        .reshape(B, OC, NCORES * LPC)[:, :, :LOUT]
    )
    return np.ascontiguousarray(out)


def kernel(x, weight, bias):
    global LAST_RESULTS
    import time

    from concourse.bass_utils import run_bass_kernel_spmd

    # attempt order: fast prepared-writeback build (29011ns), retried once
    # after a pause (a transiently wedged device recovers in ~20-60s), then
    # the plain-DMA fallback build (30491ns) which uses no extended
    # instructions at all
    attempts = [
        ("wb", dict(), 0),
        ("wb", dict(), 25),
        ("nowb", dict(wb_outs=False, bias_in_x=False, trim_exit=1,
                      out_bufs=3), 20),
    ]
    last_exc = None
    for key, build_kw, delay in attempts:
        if delay:
            time.sleep(delay)
        try:
            if _CACHE.get(key) is None:
                _CACHE[key] = _build_bass(**build_kw)
            nc = _CACHE[key]
            in_maps = _prepare_inputs(
                x, weight, bias,
                bias_in_x=build_kw.get("bias_in_x", True))
            res = run_bass_kernel_spmd(nc, in_maps,
                                       core_ids=list(range(NCORES)))
            LAST_RESULTS = res
            _CACHE["nc"] = nc  # for test.py's TimelineSim fallback
            return _assemble(res.results)
        except Exception as e:  # noqa: BLE001 - device/compile faults
            last_exc = e
    raise last_exc



# revision 51
# speedup vs baseline: 1.0014x; 1.0014x over previous
"""LocallyConnected1D Trainium2 kernel (8-core SPMD, Bass/Tile).

out[b,o,l] = sum_{i,k} x[b,i,l+k] * w[l,o,i,k] + bias[o,l]
  B=64, I=O=128, K=8, L_in=512, L_out=505 (stride 1), fp32 I/O.

Sharding: OUT_LEN across 8 cores (64 positions each, padded 505->512).
Each position is an independent GEMM: out[:, :, l] = X_l @ W_l with
contract dim I*K=1024 split into 8 accumulating 128-contract matmuls.
Weight slice [i, o] is the stationary operand (full 128x128 array),
x window [i, b] streams.

Precision: weights and x are cast to fp8 e3m4 on host (the weight DMA
is the roofline: 265MB fp32 -> 66MB fp8), PSUM accumulates fp32, bias
is added in fp32 on DVE, and the output is written back bf16 and
upcast to fp32 on host. Measured end-to-end rel err 1.68e-2 (L2) /
1.77e-2 (max, absmax-scaled) on the fixed-seed reference inputs —
under the 2e-2 gate; set x_fp8=False (bf16 x, 33.8us) for 1.20e-2.

Schedule (tuned against TimelineSim, HW-verified 30491ns = 1300 head
+ ~27921 stream + ~137 tail gaps + 900 DMA-sem epilogue + 233 exit
barrier). The stream runs at the model's full 360GB/s with zero
mid-stream gaps, so everything after the fp8 cast is tail/head work:
- weight blocks taper at both ends ((2,2,4)+(8,)*5+(4,4,2,2,2,1,1)):
  small head blocks start the PE early; the fine late taper keeps
  PE's last-16-position stretch sem-locked to each block's arrival
  instead of queued behind one big 8-block semaphore.
- the last position's weights are split 4+4 taps (split_last_tap=4,
  both 512B/partition descriptors, no sub-512B 2x penalty): after the
  final 46KB morsel's sem only 4 matmuls + one bias-add remain on the
  critical chain.
- bias-adds for positions 56-62 ride the idle Activation engine
  (nc.scalar.add) so DVE is free to run position 63's bias the moment
  its PSUM lands (DVE's 216ns/op tail queue otherwise delays it).
  GPSIMD/Pool cannot read PSUM (BIR verifier rejects it).
- out blocks (48,8,4,4) ride after the last weight bytes: out1/out2 on
  Pool SWDGE, out3+final on SP HWDGE (out3's dispatch fires on
  bias59's sem, well before the final block's chain, so SP's in-order
  sequencer never stalls the final dispatch).
- x arrives in (2+15)+8*7 column chunks (each >=512B/partition). The
  bf16 bias no longer gets its own DMA (128B/partition would pay the
  sub-512B 2x penalty): its raw bytes ride as two leading fp8 columns
  of x inside x0's single 1088B-descriptor transfer, recovered on-chip
  by a flatten+bitcast view and upcast once on Activation (DVE's
  tensor_scalar bias operand must be fp32). Saves a net 45ns of
  stream time and one SWDGE dispatch.
IR post-passes (TimelineSim and the NEFF see the same mutated IR):
the first 3 wait-free weight DMAs are hoisted above the framework
preamble (DMA pipe overlaps the ~1us engine-start rendezvous) and the
trailing exit-barrier instructions after Pool's ISA are dropped
(trim_exit=1). The deeper trim (trim_exit=2, -233ns in sim) removes
the second barrier round entirely and WEDGES the real device
(NRT_EXEC_UNIT_UNRECOVERABLE) — do not enable it.
wb_outs (default ON, HW-verified 28969ns with bias_in_x and the
exit-barrier wait-move / rel err identical to the 30718ns baseline):
all
four out blocks leave via prepared SWDGE kv_writebacks instead of
DMACopies. Each block's prep (descriptor gen, ~1us on the idle Pool
engine) is emitted after its bias-adds — so Tile defers the RAW waits
onto the trigger — then relocated early in IR with its waits moved
onto the trigger (the trigger is the actual read point; the prep
itself only needs the idxt memset, which precedes it in Pool program
order — preps MUST land after that memset or the Q7 reads garbage ctx
indices and the OOB guard silently skips every write). Blocks 1-3
fire from one combined trigger emitted just before the final prep
(their transfers ride the idle post-stream window instead of cutting
into the weight stream); the final block's trigger fires ~60ns after
bias63's sem, replacing the 650+650 dispatch+DGE chain. Each kv
"batch" writes a pow2 (<=2048B) contiguous ctx run (out1 = 3x16
positions, ncn=1024), so descriptors stay big. Making this execute on
real HW needs two Bacc passes replayed here: insert_library_loads
(kv_writeback's Q7 ucode lives in the reloadable 'attn' library —
without the MODIFY_POOL_CONFIG load the device faults unrecoverably)
and lower_extended_insts (populates extended-inst .instr bytes; else
walrus fails with "ISA wrong length"). The framework's exit
flush-drain expects the SWDGE queue sems the preps no longer post;
an IR pass clamps those waits to actual posts and waits the custom
wb_out sem instead. Remaining structure is at its floor: 1300 head
+ ~24963 input-byte stream + 900 w-sem + ~420 PE/bias chain + ~75
trigger+transfer + 900 out-sem + ~233 exit barrier. Probed and dead:
prepared-gather head start (SWDGE gen 994ns + prep-done sem lands
first bytes at ~1430 vs HWDGE's 1300), sub-8-bit weights (break the
2e-2 gate), every exit-barrier instruction REMOVAL (wedges the device; moving
the flush-drain's waits onto Pool's gather-waiter EventSemaphore with
all instructions kept is safe and saves 42ns), dma_transpose
weight loads (cost model's 14ns/16x128-tile matches byte rate and the
instruction is 2-byte-dtype-only).

kernel() retries the fast build once after a 25s pause (a transiently
wedged device — e.g. from a prior faulting tenant — usually recovers),
then falls back to a plain-DMA build (30491ns, no extended
instructions) so a degraded device still produces a correct result.
"""

import json

import numpy as np
import ml_dtypes

B = 64
IC = 128
OC = 128
KW = 8
LIN = 512
LOUT = 505
NCORES = 8
LPC = 64  # padded positions per core: 8*64 = 512 >= 505
TW = LPC + KW - 1  # x time-columns a core touches (71)
TPAD = (NCORES - 1) * LPC + TW  # padded x length (519)
OB = 8  # x-chunk width (columns) and w/out block alignment granularity

_BF16 = ml_dtypes.bfloat16
_F8 = ml_dtypes.float8_e3m4

_CACHE: dict = {}
LAST_RESULTS = None  # BassKernelResults of the most recent kernel() call


def _hoist_head_dmas_ir(nc, n: int = 2, top: bool = False) -> None:
    """Move the first `n` wait-free SP DMACopy instructions from the body
    block into the preamble block, after SP's RegisterMoves but before the
    start barrier. The DMA pipe (dispatch+HWDGE+DGE delay) then overlaps the
    ~1us engine-start rendezvous, starting the weight stream ~0.8us earlier.
    Safe because the hoisted DMAs wait on nothing, nothing reads their tiles
    until their completion semaphores fire (well after the preamble), and
    SP's own preamble order (RegisterMoves first) is preserved. Mutates the
    in-memory IR so TimelineSim and the NEFF see the same program."""
    import concourse.mybir as mybir

    blocks = nc.m.functions[0].blocks
    if len(blocks) < 2:
        return
    pre, body = blocks[0].instructions, blocks[1].instructions
    hoist = []
    for inst in body:
        if len(hoist) >= n:
            break
        si = getattr(inst, "sync_info", None)
        waits = getattr(si, "on_wait", None) if si is not None else None
        if (type(inst).__name__ == "InstDMACopy"
                and inst.engine == mybir.EngineType.SP and not waits):
            hoist.append(inst)
    if not hoist:
        return
    ids = {id(i) for i in hoist}
    kept = [i for i in body if id(i) not in ids]
    del body[:]
    body.extend(kept)
    if top:
        idx = 1  # right after the framework dummy Call
    else:
        idx = max(i for i, inst in enumerate(pre)
                  if inst.engine == mybir.EngineType.SP
                  and type(inst).__name__ == "InstRegisterMove") + 1
    for k, inst in enumerate(hoist):
        pre.insert(idx + k, inst)


def _trim_exit_barrier_ir(nc, deep: bool = False) -> None:
    """Drop the second (redundant) all-engine barrier round at program exit.
    Round 1 already rendezvouses after SP's big DMA-flush drain (the W:16
    wait on every DMA-completion semaphore), so outputs are in DRAM before
    any engine passes it; the trailing Pool ISA op is kept as the final
    instruction."""
    blocks = nc.m.functions[0].blocks
    exit_insts = blocks[-1].instructions
    isa_idx = [i for i, inst in enumerate(exit_insts)
               if type(inst).__name__ == "InstISA"]
    if not isa_idx:
        return
    if deep:
        # keep every engine's Drain/ISA teardown instructions (nrt needs
        # each engine's stream to terminate properly) but strip the
        # cross-engine barrier EventSemaphores and their waits so engines
        # finish independently instead of paying the ~233ns second
        # rendezvous round
        kept = []
        for i, inst in enumerate(exit_insts[: isa_idx[-1] + 1]):
            tn = type(inst).__name__
            if tn == "InstEventSemaphore":
                continue
            if tn in ("InstDrain", "InstISA"):
                si = getattr(inst, "sync_info", None)
                if i > 0 and si is not None and getattr(si, "on_wait", None):
                    del si.on_wait[:]
                kept.append(inst)
    else:
        if isa_idx[-1] == len(exit_insts) - 1:
            return
        kept = exit_insts[: isa_idx[-1] + 1]
    del exit_insts[:]
    exit_insts.extend(kept)


# --- workaround: this walrus build rejects >1 sync wait per instruction ----
def _split_waits(raw: bytes) -> bytes:
    m = json.loads(raw)
    ctr = 0
    for f in m.get("functions", []):
        for blk in f.get("blocks", []) or f.get("basicblocks", []):
            out = []
            for inst in blk.get("instructions", []):
                si = inst.get("sync_info")
                waits = (si or {}).get("on_wait") or []
                if len(waits) > 1:
                    for w in waits[:-1]:
                        ctr += 1
                        out.append(
                            {
                                "debug": inst.get("debug", 0),
                                "engine": inst["engine"],
                                "ins": [],
                                "name": f"waitsplit_{ctr}",
                                "opcode": "EventSemaphore",
                                "outs": [],
                                "sync_info": {"on_update": [], "on_wait": [w]},
                            }
                        )
                    si["on_wait"] = waits[-1:]
                out.append(inst)
            blk["instructions"] = out
    return json.dumps(m).encode()


def _build_bass(w_bufs: int = 3, psum_bufs: int = 8, out_bufs: int = 4,
                w_sched=(2, 2, 4) + (8,) * 5 + (4, 4, 2, 2, 2, 1, 1),
                out_sched=(48, 8, 4, 4),
                xa_cols: int = 15, out_eng: str = "gpsimd",
                bias_eng: str = "gpsimd", x0_eng: str = "gpsimd",
                x_eng: str = "sync", final_out_eng: str | None = "sync",
                sync_last_n_outs: int = 1, split_last_tap: int = 4,
                act_bias_last_n: int = 0,
                out_engs=("gpsimd", "gpsimd", "sync", "sync"),
                act_bias_ranges=(),
                bias_engs=((56, 63, "scalar"),),
                split_prefetch: int = 1, bias_in_x: bool = True,
                x_prefetch_all: bool = False, x_fp8: bool = True,
                hoist_head: int = 3, hoist_top: bool = True,
                trim_exit: int = 1, scatter_final: bool = False,
                wb_final: bool = False, wb_outs: bool = True,
                wb_defer_trigger: bool = False,
                reps: int = 1):
    import contextlib

    import concourse.bass as bass
    import concourse.tile as tile
    import concourse.mybir as mybir

    sched = list(w_sched)
    assert sum(sched) == LPC
    osched = list(out_sched)
    assert sum(osched) == LPC
    # w blocks must not straddle out blocks
    obounds = [0]
    for nb in osched:
        obounds.append(obounds[-1] + nb)
    acc = 0
    for nb in sched:
        assert any(a <= acc and acc + nb <= b
                   for a, b in zip(obounds[:-1], obounds[1:]))
        acc += nb

    # x column chunks: [0, xa_cols) then OB-wide chunks to TW
    xbounds = [0, xa_cols]
    while xbounds[-1] < TW:
        xbounds.append(min(xbounds[-1] + OB, TW))

    xdt = mybir.dt.float8e3 if x_fp8 else mybir.dt.bfloat16

    if wb_outs:
        wb_final = True
    nc = bass.Bass(num_swdge_queues=2 if wb_final else 1)
    # bias_in_x: the 128 bias bytes per partition ride as two extra fp8
    # columns at the HEAD of x (cols 0-1), so they move inside x0's single
    # >=512B-descriptor transfer instead of a separate 128B/partition DMA
    # that pays the sub-512B 2x latency penalty (91ns -> +45ns net save)
    xcols = TW + 2 if bias_in_x else TW
    x_d = nc.dram_tensor("x", [IC, xcols, B], xdt, kind="ExternalInput")
    w_d = nc.dram_tensor(
        "w", [IC, LPC, KW, OC], mybir.dt.float8e3, kind="ExternalInput"
    )
    if not bias_in_x:
        b_d = nc.dram_tensor("bias", [OC, LPC], mybir.dt.bfloat16,
                             kind="ExternalInput")
    if scatter_final:
        ix_d = nc.dram_tensor("idx", [128, 8], mybir.dt.int16,
                              kind="ExternalInput")
    o_d = nc.dram_tensor("out", [OC, LPC, B], mybir.dt.bfloat16, kind="ExternalOutput")

    # out DMAs go on their own queue: their compute-dependency waits must not
    # block later weight-block DMAs behind them on SP's in-order sequencer
    oeng = getattr(nc, out_eng)

    with tile.TileContext(nc) as tc:
        with (
            tc.tile_pool(name="const", bufs=1) as constp,
            tc.tile_pool(name="wp", bufs=w_bufs) as wp,
            tc.tile_pool(name="op", bufs=out_bufs) as op,
            tc.tile_pool(name="ps", bufs=psum_bufs, space="PSUM") as pp,
        ):
            # x chunk tiles; chunk 0 lands first so the PE can start early
            xtiles = []  # (start_col, tile)
            nchunks = len(xbounds) - 1
            xoff = 2 if bias_in_x else 0  # bias cols precede x col 0
            for ci in range(nchunks):
                c0, c1 = xbounds[ci], xbounds[ci + 1]
                w_extra = xoff if ci == 0 else 0
                xt = constp.tile([IC, c1 - c0 + w_extra, B], xdt,
                                 name=f"x{ci}", tag=f"x{ci}")
                xtiles.append((c0, xt))
            xdma_done = [False] * nchunks

            def need_x(col):
                ci = next(i for i in range(nchunks)
                          if xbounds[i] <= col < xbounds[i + 1])
                if not xdma_done[ci]:
                    c0, xt = xtiles[ci]
                    d0 = 0 if ci == 0 else c0 + xoff
                    eng = getattr(nc, x0_eng if ci == 0 else x_eng)
                    eng.dma_start(xt[:], x_d[:, d0: d0 + xt.shape[1]])
                    xdma_done[ci] = True
                return ci

            def x_ap(col):
                ci = need_x(col)
                c0, xt = xtiles[ci]
                return xt[:, col - c0 + (xoff if ci == 0 else 0), :]

            need_x(0)
            if bias_in_x:
                # recover the bf16 bias from x0's leading two byte-columns
                bth = (xtiles[0][1][:, 0:2, :]
                       .rearrange("p a b -> p (a b)")
                       .bitcast(mybir.dt.bfloat16))
            else:
                bth = constp.tile([OC, LPC], mybir.dt.bfloat16)
                getattr(nc, bias_eng).dma_start(bth[:], b_d[:])
            fnb = osched[-1]
            ot_final = None
            wb_sem = None
            if wb_final:
                # final out block leaves via a prepared SWDGE kv_writeback:
                # descriptors are generated early on Pool; at the tail a
                # ~60ns trigger_dma (no dispatch/HWDGE/DGE chain) fires the
                # transfer as soon as the last bias-add lands. The whole
                # [OC, fnb, B] block is written as ONE kv "batch" with a
                # (fnb*B)-element contiguous ctx run -> 512B descriptors,
                # dodging the sub-512B 2x penalty.
                def wb_geom(onb):
                    p = 1
                    while p * 2 <= onb and onb % (p * 2) == 0 \
                            and (p * 2) * B <= 2048:
                        p *= 2
                    return onb // p, p  # (kv batch, positions per batch)

                max_bt = max(wb_geom(nb)[0] for nb in osched) if wb_outs else 1
                idxt = constp.tile([128, max_bt], mybir.dt.int32, name="wbidx")
                # memset on Pool: program order guarantees it precedes the
                # (wait-stripped, relocated) preps on the same sequencer
                nc.gpsimd.memset(idxt[:], 0)
                sem_ctx = nc.semaphore("wb_out")
                wb_sem = sem_ctx.__enter__()
                wb_expect = 0

                def wb_view(ol0, onb):
                    bt, p = wb_geom(onb)
                    return (o_d[:, ol0: ol0 + onb, :]
                            .rearrange("(oi oo) (bt li) b -> bt oi oo (li b)",
                                       oo=1, bt=bt))

                fbt, fp = wb_geom(fnb)
                ot_final = op.tile([OC, 1, fbt, fp * B], mybir.dt.bfloat16,
                                   name="otf", tag="otf")
                wb_out_ap = wb_view(LPC - fnb, fnb)
            if scatter_final:
                # final out block goes out via a pre-prepared SWDGE scatter:
                # descriptors are generated early; at the tail only a ~40ns
                # trigger separates the last bias-add from the transfer,
                # replacing the 153+625+650 dispatch+HWDGE+DGE-delay chain.
                # scatter ADDs, so zero the target DRAM region first (early).
                zt = constp.tile([OC, fnb * B], mybir.dt.bfloat16, name="zt")
                nc.vector.memset(zt[:], 0)
                nc.sync.dma_start(o_d[:, LPC - fnb:, :], zt[:])
                idxt = constp.tile([128, 8], mybir.dt.int16, name="idxt")
                nc.sync.dma_start(idxt[:], ix_d[:])
                ot_final = op.tile([OC, 1, fnb * B], mybir.dt.bfloat16,
                                   name="otf", tag="otf")
                import contextlib as _cl
                sem_ctx = nc.semaphore("scat_out")
                scat_sem = sem_ctx.__enter__()
                nc.gpsimd.dma_scatter_add(
                    out_ap=o_d[:, LPC - fnb:, :].opt({0}),
                    in_ap=ot_final[:],
                    idxs_ap=idxt[:],
                    num_idxs=128,
                    num_idxs_reg=128,
                    elem_size=fnb * B,
                    elem_step=LPC * B,
                    prepare_only=True,
                    sem=scat_sem,
                )

            if x_prefetch_all is True:
                # stream order doesn't change when the last w block lands
                # (pool is serial, bytes are bytes), but early x makes every
                # tail-position x-semaphore long-satisfied by drain time
                for ci in range(nchunks):
                    need_x(xbounds[ci])
            # tensor_scalar_add needs an fp32 scalar operand: upcast once on
            # the (otherwise idle) Activation engine, off the critical path
            bt = constp.tile([OC, LPC], mybir.dt.float32)
            nc.scalar.copy(bt[:], bth if bias_in_x else bth[:])

            if reps > 1:  # timing mode: hoist x loads out of the repeat loop
                for ci in range(nchunks):
                    need_x(xbounds[ci])

            blocks = []  # (l0, nb)
            l0 = 0
            for nb in sched:
                blocks.append((l0, nb))
                l0 += nb

            rep_ctx = tc.For_i(0, reps, 1) if reps > 1 else contextlib.nullcontext()
            with rep_ctx:
                bi = 0  # next block to process
                wt = None
                wt_tap = None
                wl0 = wnb = 0
                for ol0, onb in zip(obounds[:-1], osched):
                    is_final = ol0 + onb == LPC
                    wb_block = wb_outs or (wb_final and is_final)
                    if (scatter_final or wb_final) and is_final:
                        ot = ot_final
                        obt, opp = (fbt, fp) if wb_final else (1, onb)
                    elif wb_block:
                        obt, opp = wb_geom(onb)
                        ot = op.tile([OC, 1, obt, opp * B],
                                     mybir.dt.bfloat16,
                                     name=f"ot{onb}", tag=f"ot{onb}")
                    else:
                        ot = op.tile([OC, onb, B], mybir.dt.bfloat16,
                                     name=f"ot{onb}", tag=f"ot{onb}")
                    for j in range(onb):
                        l = ol0 + j
                        if wt is None or l >= wl0 + wnb:
                            wl0, wnb = blocks[bi]
                            bi += 1
                            do_split = (split_last_tap
                                        and blocks[-1][1] == 1)
                            if do_split and bi == len(blocks):
                                # final position: leading taps were prefetched
                                # a block early (below); only the last
                                # split_last_tap taps arrive last, shrinking
                                # the post-arrival critical chain
                                nt = int(split_last_tap)
                                wt = wtf_a
                                wt_tap = wp.tile([IC, 1, nt, OC],
                                                 mybir.dt.float8e3,
                                                 name="wtf_b", tag="wtf_b")
                                nc.sync.dma_start(
                                    wt_tap[:], w_d[:, wl0: wl0 + 1, KW - nt:])
                            else:
                                wt = wp.tile([IC, wnb, KW, OC],
                                             mybir.dt.float8e3,
                                             name=f"wt{wnb}", tag=f"wt{wnb}")
                                wt_tap = None
                                # prefetch x chunks this block touches first
                                need_x(wl0 + wnb - 1 + KW - 1)
                                nc.sync.dma_start(wt[:], w_d[:, wl0: wl0 + wnb])
                            if (isinstance(x_prefetch_all, int)
                                    and x_prefetch_all is not True
                                    and x_prefetch_all > 0
                                    and bi == x_prefetch_all):
                                # deferred full-x prefetch: PE has ramped on
                                # the early blocks; remaining x rides now so
                                # tail x-semaphores are long satisfied
                                for ci in range(nchunks):
                                    need_x(xbounds[ci])
                            if do_split and bi == len(blocks) - split_prefetch:
                                nt = int(split_last_tap)
                                fl0 = blocks[-1][0]
                                wtf_a = wp.tile([IC, 1, KW - nt, OC],
                                                mybir.dt.float8e3,
                                                name="wtf_a", tag="wtf_a")
                                need_x(fl0 + KW - 1)
                                nc.sync.dma_start(
                                    wtf_a[:], w_d[:, fl0: fl0 + 1, : KW - nt])
                        ps = pp.tile([OC, B], mybir.dt.float32)
                        for k in range(KW):
                            if wt_tap is not None and k >= KW - int(split_last_tap):
                                src = wt_tap[:, l - wl0, k - (KW - int(split_last_tap)), :]
                            else:
                                src = wt[:, l - wl0, k, :]
                            nc.tensor.matmul(
                                ps[:],
                                src,
                                x_ap(l + k),
                                start=(k == 0),
                                stop=(k == KW - 1),
                            )
                        if scatter_final and is_final:
                            nc.vector.tensor_scalar_add(
                                ot_final[:, 0, j * B: (j + 1) * B],
                                ps[:], bt[:, l: l + 1]
                            )
                        elif wb_block:
                            jb, jl = divmod(j, opp)
                            tgt = ot[:, 0, jb, jl * B: (jl + 1) * B]
                            weng = next(
                                (e for a, b, e in bias_engs if a <= l < b),
                                "vector")
                            if weng == "scalar":
                                nc.scalar.add(tgt, ps[:], bt[:, l: l + 1])
                            else:
                                getattr(nc, weng).tensor_scalar_add(
                                    tgt, ps[:], bt[:, l: l + 1])
                        elif (beng := next(
                                (e for a, b, e in bias_engs if a <= l < b),
                                "scalar" if (l >= LPC - act_bias_last_n
                                             or any(a <= l < b
                                                    for a, b in act_bias_ranges))
                                else None)) is not None:
                            # tail positions: bias-add off DVE (Activation's
                            # add or Pool's tensor_scalar_add) to dodge DVE's
                            # 216ns/op tail queue
                            if beng == "scalar":
                                nc.scalar.add(ot[:, j, :], ps[:],
                                              bt[:, l: l + 1])
                            else:
                                getattr(nc, beng).tensor_scalar_add(
                                    ot[:, j, :], ps[:], bt[:, l: l + 1])
                        else:
                            nc.vector.tensor_scalar_add(
                                ot[:, j, :], ps[:], bt[:, l: l + 1]
                            )
                    if scatter_final and is_final:
                        nc.gpsimd.trigger_dma(count=None)
                        nc.gpsimd.wait_ge(scat_sem, 1)
                        continue
                    if wb_block:
                        if is_final:
                            # fire blocks 1..3's prepared writebacks now
                            # (one combined trigger, gated by their bias
                            # sems via the preps' deferred deps): their
                            # transfers ride the idle post-stream window
                            # instead of cutting into the weight stream
                            nc.gpsimd.trigger_dma(count=None, queue_num=1)
                        # emitted after the bias-adds so Tile defers the
                        # prep's RAW waits onto the trigger (prep itself is
                        # wait-free and gets hoisted early in IR below)
                        nc.gpsimd.kv_writeback(
                            out_ap=(wb_out_ap if is_final
                                    else wb_view(ol0, onb)),
                            in_ap=ot[:],
                            ctx_idxs_ap=idxt[:, :obt],
                            prepare_only=True,
                            sem=wb_sem,
                            queue_num=1,
                        )
                        wb_expect += 16
                        if is_final:
                            nc.gpsimd.trigger_dma(count=None, queue_num=1)
                        continue
                    oidx = obounds.index(ol0)
                    if out_engs is not None:
                        eng = getattr(nc, out_engs[oidx])
                    else:
                        eng = oeng
                        if final_out_eng is not None and oidx >= len(osched) - sync_last_n_outs:
                            eng = getattr(nc, final_out_eng)
                    eng.dma_start(o_d[:, ol0: ol0 + onb, :], ot[:])

    if scatter_final or wb_final:
        # Tile sinks the prepare next to its trigger at the program tail,
        # putting the ~1us SWDGE descriptor generation on the critical chain
        # (and starving the trigger's no_exec FIFO visit). Move it early: its
        # only wait is the idx tile (memset/DMA, ~2us); parking Pool's
        # sequencer on that is harmless since the next Pool work (out
        # dispatches) is much later.
        prep_ty = ("InstDMAScatterAddAnt" if scatter_final
                   else "InstKVWritebackAnt")
        body = nc.m.functions[0].blocks[1].instructions
        preps = [inst for inst in body if type(inst).__name__ == prep_ty]
        prep = preps[0]
        if wb_final:
            # The prep's data read happens at trigger time, but the emitted
            # sync waits (on the bias-adds that fill its source tile) sit on
            # the prep and would park Pool's sequencer until the tail. MOVE
            # each prep's waits onto its trigger — the trigger is the actual
            # read point, so the data dependency stays sound while the prep
            # (descriptor gen only) runs early. The prep's remaining dep, the
            # idxt memset, precedes it in Pool program order.
            for _p in preps:
                _si = getattr(_p, "sync_info", None)
                if _si is None or not getattr(_si, "on_wait", None):
                    continue
                _trig = None
                _seen = False
                for _inst in body:
                    if _inst is _p:
                        _seen = True
                    elif _seen and type(_inst).__name__ == "InstTriggerDma":
                        _trig = _inst
                        break
                assert _trig is not None, "prep without trigger"
                _tsi = _trig.sync_info
                _have = {(w.id, w.wait_mode): w
                         for w in (_tsi.on_wait or [])}
                for w in _si.on_wait:
                    k = (w.id, w.wait_mode)
                    if k in _have:
                        _have[k].wait_value = max(
                            _have[k].wait_value or 0, w.wait_value or 0)
                    else:
                        _tsi.on_wait.append(w)
                        _have[k] = w
                del _si.on_wait[:]
        if True:
            kept = [inst for inst in body
                    if type(inst).__name__ != prep_ty]
            del body[:]
            body.extend(kept)
        import concourse.mybir as _mb
        n_pool = 0
        ins_at = 0
        for i, inst in enumerate(body):
            if (inst.engine == _mb.EngineType.Pool
                    and type(inst).__name__ == "InstDMACopy"):
                n_pool += 1
                if n_pool == 2:  # after x0 and bias dispatches
                    ins_at = i + 1
                    break
        if wb_final:
            # the preps' descriptor-gen READS the idxt tile at gen time:
            # they must land after its Pool memset (and after the library
            # reload), or the Q7 reads garbage ctx indices and the OOB
            # guard silently skips every write
            for i in range(ins_at, len(body)):
                inst = body[i]
                if (inst.engine == _mb.EngineType.Pool
                        and type(inst).__name__ == "InstMemset"):
                    ins_at = i + 1
                    break
        for k, _p in enumerate(preps):
            body.insert(ins_at + k, _p)
    if wb_final:
        # The framework's exit flush-drain waits every DMA-queue sem to its
        # expected final value, but the prepared writeback posts its 16
        # completion bumps to the custom wb_out sem instead of its SWDGE
        # queue's builtin sem. Clamp each drain wait to what the program
        # actually posts (drop if nothing does) and wait the wb sem
        # explicitly so the drain still covers the writeback's landing.
        posted: dict[int, int] = {}
        wb_id = None
        for blk in nc.m.functions[0].blocks:
            for inst in blk.instructions:
                si = getattr(inst, "sync_info", None)
                for upd in (getattr(si, "on_update", None) or []):
                    if str(getattr(upd, "update_mode", "")) in (
                            "sem-inc", "sem-add-imm"):
                        posted[upd.id] = posted.get(upd.id, 0) + (
                            upd.update_value or 0)
                        if (upd.ant_name or "").startswith("wb_out"):
                            wb_id = upd.id
        import concourse.mybir as _mb2
        for inst in nc.m.functions[0].blocks[-1].instructions:
            si = getattr(inst, "sync_info", None)
            waits = (getattr(si, "on_wait", None) or [])
            if not waits or type(inst).__name__ != "InstDrain":
                continue
            new_waits = []
            patched = False
            for w in waits:
                nm = w.ant_name or ""
                if (nm.startswith("DMASW") or nm.startswith("DMAHW")):
                    have = posted.get(w.id, 0)
                    if have <= 0:
                        patched = True
                        continue  # nothing posts: unsatisfiable, drop
                    if have < (w.wait_value or 0):
                        w.wait_value = have
                        patched = True
                new_waits.append(w)
            if patched and wb_id is not None:
                new_waits.append(_mb2.SyncWait(
                    sync_type="semaphore", id=wb_id, ant_name="wb_out",
                    wait_mode="sem-ge-imm",
                    wait_value=posted.get(wb_id, 16), wait_reg=None))
            if patched:
                del si.on_wait[:]
                si.on_wait.extend(new_waits)
    if wb_final:
        # Move the exit flush-drain's wait list onto Pool's gather-waiter
        # EventSemaphore: every instruction and the barrier protocol stay
        # intact, but SP (whose body ends early now that the final out
        # dispatch is gone) arrives at the barrier immediately instead of
        # serializing arrival+propagation after the wb semaphore. Pool's
        # waiter then fires directly on the last DMA completion, trimming
        # the post-drain chain.
        _ex = nc.m.functions[0].blocks[-1].instructions
        _flush = _ex[0]
        _fsi = getattr(_flush, "sync_info", None)
        if (type(_flush).__name__ == "InstDrain" and _fsi is not None
                and len(_fsi.on_wait or []) > 4):
            _tgt = next(
                (i for i in _ex
                 if type(i).__name__ == "InstEventSemaphore"
                 and str(i.engine).endswith("Pool")
                 and any("gather" in (w.ant_name or "")
                         for w in (i.sync_info.on_wait or []))),
                None)
            if _tgt is not None:
                _tsi = _tgt.sync_info
                _moved = list(_fsi.on_wait)
                # keep the latest-firing wait (wb_out) last on the
                # instruction itself; earlier ones become cheap pre-waits
                _moved.sort(key=lambda w: (w.ant_name or "") == "wb_out")
                _keep = list(_tsi.on_wait)
                del _tsi.on_wait[:]
                _tsi.on_wait.extend(_keep + _moved)
                del _fsi.on_wait[:]
    if hoist_head > 0:
        _hoist_head_dmas_ir(nc, n=hoist_head, top=hoist_top)
    if trim_exit:
        _trim_exit_barrier_ir(nc, deep=(trim_exit == 2))
    if wb_final or scatter_final:
        # kv_writeback's Q7 ucode lives in the reloadable 'attn' library,
        # not the boot default. Bacc.compile runs the insert_library_loads
        # pass to place MODIFY_POOL_CONFIG loads before instructions that
        # need a non-resident library (and back-switches for standard-lib
        # ops); raw Bass never does, so run the same rust pass here.
        from concourse.library_config import all_libraries, standard
        import bass_rust as _br
        _mask: dict = {}
        for _lib in all_libraries:
            for _it in _lib.instructions:
                _mask[_it] = _mask.get(_it, 0) | (1 << _lib.index)
        _br.insert_library_loads(nc, _mask, len(all_libraries),
                                 standard.index)
        # extended insts (kv_writeback / trigger_dma / IncSwdgeSem) carry
        # their ISA encodings in .instr — raw Bass never populates them
        # (Bacc.compile does); without this the NEFF codegen fails with
        # "ISA wrong length"
        from concourse.library_overlay import lower_extended_insts
        lower_extended_insts(nc)
    fixed = _split_waits(bass.Bass.to_json_bytes(nc))
    nc.to_json_bytes = lambda: fixed  # type: ignore[method-assign]
    return nc


def _prepare_inputs(x, weight, bias, x_fp8=True, bias_in_x=True):
    x = np.asarray(x, dtype=np.float32)
    weight = np.asarray(weight, dtype=np.float32)
    bias = np.asarray(bias, dtype=np.float32)

    # x: [b, i, t] -> bf16/fp8, pad t to TPAD, transpose -> [i, t, b]
    xdt = _F8 if x_fp8 else _BF16
    xpad = np.zeros((B, IC, TPAD), dtype=xdt)
    xpad[:, :, :LIN] = x.astype(xdt)
    xt = xpad.transpose(1, 2, 0)  # [i, t, b] view

    # weight: [l, o, i, k] -> fp8 e3m4, pad l, transpose -> [i, l, k, o]
    wpad = np.zeros((NCORES * LPC, OC, IC, KW), dtype=_F8)
    wpad[:LOUT] = weight.astype(_F8)
    wt = wpad.transpose(2, 0, 3, 1)  # [i, l, k, o] view

    bpad = np.zeros((OC, NCORES * LPC), dtype=_BF16)
    bpad[:, :LOUT] = bias.astype(_BF16)

    in_maps = []
    for c in range(NCORES):
        l0 = c * LPC
        if bias_in_x:
            # bias [OC, LPC] bf16 -> 128 raw bytes per partition -> two
            # leading fp8 byte-columns of x (matches the on-chip bitcast:
            # free-dim-contiguous little-endian bf16 pairs)
            bb = (np.ascontiguousarray(bpad[:, l0: l0 + LPC])
                  .view(np.uint8)          # [OC, 2*LPC]
                  .reshape(IC, 2, B)
                  .view(xdt if x_fp8 else np.uint8))
            if not x_fp8:
                raise NotImplementedError("bias_in_x requires x_fp8")
            xc = np.concatenate(
                [bb, np.ascontiguousarray(xt[:, l0: l0 + TW, :])], axis=1)
            in_maps.append(
                {
                    "x": np.ascontiguousarray(xc),
                    "w": np.ascontiguousarray(wt[:, l0: l0 + LPC]),
                }
            )
        else:
            in_maps.append(
                {
                    "x": np.ascontiguousarray(xt[:, l0: l0 + TW, :]),
                    "w": np.ascontiguousarray(wt[:, l0: l0 + LPC]),
                    "bias": np.ascontiguousarray(bpad[:, l0: l0 + LPC]),
                }
            )
    return in_maps


def _assemble(results):
    full = np.stack([results[c]["out"] for c in range(NCORES)], axis=0)
    # [c, o, l_loc, b] (bf16) -> fp32 [b, o, c*LPC + l_loc] -> crop to LOUT
    out = (
        full.astype(np.float32)
        .transpose(3, 1, 0, 2)
        .reshape(B, OC, NCORES * LPC)[:, :, :LOUT]
    )
    return np.ascontiguousarray(out)


def kernel(x, weight, bias):
    global LAST_RESULTS
    import time

    from concourse.bass_utils import run_bass_kernel_spmd

    # attempt order: fast prepared-writeback build (29011ns), retried once
    # after a pause (a transiently wedged device recovers in ~20-60s), then
    # the plain-DMA fallback build (30491ns) which uses no extended
    # instructions at all
    attempts = [
        ("wb", dict(), 0),
        ("wb", dict(), 25),
        ("nowb", dict(wb_outs=False, bias_in_x=False, trim_exit=1,
                      out_bufs=3), 20),
    ]
    last_exc = None
    for key, build_kw, delay in attempts:
        if delay:
            time.sleep(delay)
        try:
            if _CACHE.get(key) is None:
                _CACHE[key] = _build_bass(**build_kw)
            nc = _CACHE[key]
            in_maps = _prepare_inputs(
                x, weight, bias,
                bias_in_x=build_kw.get("bias_in_x", True))
            res = run_bass_kernel_spmd(nc, in_maps,
                                       core_ids=list(range(NCORES)))
            LAST_RESULTS = res
            _CACHE["nc"] = nc  # for test.py's TimelineSim fallback
            return _assemble(res.results)
        except Exception as e:  # noqa: BLE001 - device/compile faults
            last_exc = e
    raise last_exc



# revision 54
# speedup vs baseline: 1.0069x; 1.0055x over previous
"""LocallyConnected1D Trainium2 kernel (8-core SPMD, Bass/Tile).

out[b,o,l] = sum_{i,k} x[b,i,l+k] * w[l,o,i,k] + bias[o,l]
  B=64, I=O=128, K=8, L_in=512, L_out=505 (stride 1), fp32 I/O.

Sharding: OUT_LEN across 8 cores (64 positions each, padded 505->512).
Each position is an independent GEMM: out[:, :, l] = X_l @ W_l with
contract dim I*K=1024 split into 8 accumulating 128-contract matmuls.
Weight slice [i, o] is the stationary operand (full 128x128 array),
x window [i, b] streams.

Precision: weights and x are cast to fp8 e3m4 on host (the weight DMA
is the roofline: 265MB fp32 -> 66MB fp8), PSUM accumulates fp32, bias
is added in fp32 on DVE, and the output is written back bf16 and
upcast to fp32 on host. Measured end-to-end rel err 1.68e-2 (L2) /
1.77e-2 (max, absmax-scaled) on the fixed-seed reference inputs —
under the 2e-2 gate; set x_fp8=False (bf16 x, 33.8us) for 1.20e-2.

Schedule (tuned against TimelineSim, HW-verified 30491ns = 1300 head
+ ~27921 stream + ~137 tail gaps + 900 DMA-sem epilogue + 233 exit
barrier). The stream runs at the model's full 360GB/s with zero
mid-stream gaps, so everything after the fp8 cast is tail/head work:
- weight blocks taper at both ends ((2,2,4)+(8,)*5+(4,4,2,2,2,1,1)):
  small head blocks start the PE early; the fine late taper keeps
  PE's last-16-position stretch sem-locked to each block's arrival
  instead of queued behind one big 8-block semaphore.
- the last position's weights are split 4+4 taps (split_last_tap=4,
  both 512B/partition descriptors, no sub-512B 2x penalty): after the
  final 46KB morsel's sem only 4 matmuls + one bias-add remain on the
  critical chain.
- bias-adds for positions 56-62 ride the idle Activation engine
  (nc.scalar.add) so DVE is free to run position 63's bias the moment
  its PSUM lands (DVE's 216ns/op tail queue otherwise delays it).
  GPSIMD/Pool cannot read PSUM (BIR verifier rejects it).
- out blocks (48,8,4,4) ride after the last weight bytes: out1/out2 on
  Pool SWDGE, out3+final on SP HWDGE (out3's dispatch fires on
  bias59's sem, well before the final block's chain, so SP's in-order
  sequencer never stalls the final dispatch).
- x arrives in (2+15)+8*7 column chunks (each >=512B/partition). The
  bf16 bias no longer gets its own DMA (128B/partition would pay the
  sub-512B 2x penalty): its raw bytes ride as two leading fp8 columns
  of x inside x0's single 1088B-descriptor transfer, recovered on-chip
  by a flatten+bitcast view and upcast once on Activation (DVE's
  tensor_scalar bias operand must be fp32). Saves a net 45ns of
  stream time and one SWDGE dispatch.
IR post-passes (TimelineSim and the NEFF see the same mutated IR):
the first 3 wait-free weight DMAs are hoisted above the framework
preamble (DMA pipe overlaps the ~1us engine-start rendezvous) and the
trailing exit-barrier instructions after Pool's ISA are dropped
(trim_exit=1). The deeper trim (trim_exit=2, -233ns in sim) removes
the second barrier round entirely and WEDGES the real device
(NRT_EXEC_UNIT_UNRECOVERABLE) — do not enable it.
wb_outs (default ON, HW-verified 28811ns with bias_in_x and the
exit waits moved onto the final Pool ISA / rel err identical to the
30718ns baseline):
all
four out blocks leave via prepared SWDGE kv_writebacks instead of
DMACopies. Each block's prep (descriptor gen, ~1us on the idle Pool
engine) is emitted after its bias-adds — so Tile defers the RAW waits
onto the trigger — then relocated early in IR with its waits moved
onto the trigger (the trigger is the actual read point; the prep
itself only needs the idxt memset, which precedes it in Pool program
order — preps MUST land after that memset or the Q7 reads garbage ctx
indices and the OOB guard silently skips every write). Blocks 1-3
fire from one combined trigger emitted just before the final prep
(their transfers ride the idle post-stream window instead of cutting
into the weight stream); the final block's trigger fires ~60ns after
bias63's sem, replacing the 650+650 dispatch+DGE chain. Each kv
"batch" writes a pow2 (<=2048B) contiguous ctx run (out1 = 3x16
positions, ncn=1024), so descriptors stay big. Making this execute on
real HW needs two Bacc passes replayed here: insert_library_loads
(kv_writeback's Q7 ucode lives in the reloadable 'attn' library —
without the MODIFY_POOL_CONFIG load the device faults unrecoverably)
and lower_extended_insts (populates extended-inst .instr bytes; else
walrus fails with "ISA wrong length"). The framework's exit
flush-drain expects the SWDGE queue sems the preps no longer post;
an IR pass clamps those waits to actual posts and waits the custom
wb_out sem instead. Remaining structure is at its floor: 1300 head
+ ~24963 input-byte stream + 900 w-sem + ~420 PE/bias chain + ~75
trigger+transfer + 900 out-sem + ~233 exit barrier. Probed and dead:
prepared-gather head start (SWDGE gen 994ns + prep-done sem lands
first bytes at ~1430 vs HWDGE's 1300), sub-8-bit weights (break the
2e-2 gate), every exit-barrier instruction REMOVAL (wedges the device; moving
the flush-drain's waits onto Pool's gather-waiter EventSemaphore with
all instructions kept is safe and saves 42ns), dma_transpose
weight loads (cost model's 14ns/16x128-tile matches byte rate and the
instruction is 2-byte-dtype-only).

kernel() retries the fast build once after a 25s pause (a transiently
wedged device — e.g. from a prior faulting tenant — usually recovers),
then falls back to a plain-DMA build (30491ns, no extended
instructions) so a degraded device still produces a correct result.
"""

import json

import numpy as np
import ml_dtypes

B = 64
IC = 128
OC = 128
KW = 8
LIN = 512
LOUT = 505
NCORES = 8
LPC = 64  # padded positions per core: 8*64 = 512 >= 505
TW = LPC + KW - 1  # x time-columns a core touches (71)
TPAD = (NCORES - 1) * LPC + TW  # padded x length (519)
OB = 8  # x-chunk width (columns) and w/out block alignment granularity

_BF16 = ml_dtypes.bfloat16
_F8 = ml_dtypes.float8_e3m4

_CACHE: dict = {}
LAST_RESULTS = None  # BassKernelResults of the most recent kernel() call


def _hoist_head_dmas_ir(nc, n: int = 2, top: bool = False) -> None:
    """Move the first `n` wait-free SP DMACopy instructions from the body
    block into the preamble block, after SP's RegisterMoves but before the
    start barrier. The DMA pipe (dispatch+HWDGE+DGE delay) then overlaps the
    ~1us engine-start rendezvous, starting the weight stream ~0.8us earlier.
    Safe because the hoisted DMAs wait on nothing, nothing reads their tiles
    until their completion semaphores fire (well after the preamble), and
    SP's own preamble order (RegisterMoves first) is preserved. Mutates the
    in-memory IR so TimelineSim and the NEFF see the same program."""
    import concourse.mybir as mybir

    blocks = nc.m.functions[0].blocks
    if len(blocks) < 2:
        return
    pre, body = blocks[0].instructions, blocks[1].instructions
    hoist = []
    for inst in body:
        if len(hoist) >= n:
            break
        si = getattr(inst, "sync_info", None)
        waits = getattr(si, "on_wait", None) if si is not None else None
        if (type(inst).__name__ == "InstDMACopy"
                and inst.engine == mybir.EngineType.SP and not waits):
            hoist.append(inst)
    if not hoist:
        return
    ids = {id(i) for i in hoist}
    kept = [i for i in body if id(i) not in ids]
    del body[:]
    body.extend(kept)
    if top:
        idx = 1  # right after the framework dummy Call
    else:
        idx = max(i for i, inst in enumerate(pre)
                  if inst.engine == mybir.EngineType.SP
                  and type(inst).__name__ == "InstRegisterMove") + 1
    for k, inst in enumerate(hoist):
        pre.insert(idx + k, inst)


def _trim_exit_barrier_ir(nc, deep: bool = False) -> None:
    """Drop the second (redundant) all-engine barrier round at program exit.
    Round 1 already rendezvouses after SP's big DMA-flush drain (the W:16
    wait on every DMA-completion semaphore), so outputs are in DRAM before
    any engine passes it; the trailing Pool ISA op is kept as the final
    instruction."""
    blocks = nc.m.functions[0].blocks
    exit_insts = blocks[-1].instructions
    isa_idx = [i for i, inst in enumerate(exit_insts)
               if type(inst).__name__ == "InstISA"]
    if not isa_idx:
        return
    if deep:
        # keep every engine's Drain/ISA teardown instructions (nrt needs
        # each engine's stream to terminate properly) but strip the
        # cross-engine barrier EventSemaphores and their waits so engines
        # finish independently instead of paying the ~233ns second
        # rendezvous round
        kept = []
        for i, inst in enumerate(exit_insts[: isa_idx[-1] + 1]):
            tn = type(inst).__name__
            if tn == "InstEventSemaphore":
                continue
            if tn in ("InstDrain", "InstISA"):
                si = getattr(inst, "sync_info", None)
                if i > 0 and si is not None and getattr(si, "on_wait", None):
                    del si.on_wait[:]
                kept.append(inst)
    else:
        if isa_idx[-1] == len(exit_insts) - 1:
            return
        kept = exit_insts[: isa_idx[-1] + 1]
    del exit_insts[:]
    exit_insts.extend(kept)


# --- workaround: this walrus build rejects >1 sync wait per instruction ----
def _split_waits(raw: bytes) -> bytes:
    m = json.loads(raw)
    ctr = 0
    for f in m.get("functions", []):
        for blk in f.get("blocks", []) or f.get("basicblocks", []):
            out = []
            for inst in blk.get("instructions", []):
                si = inst.get("sync_info")
                waits = (si or {}).get("on_wait") or []
                if len(waits) > 1:
                    for w in waits[:-1]:
                        ctr += 1
                        out.append(
                            {
                                "debug": inst.get("debug", 0),
                                "engine": inst["engine"],
                                "ins": [],
                                "name": f"waitsplit_{ctr}",
                                "opcode": "EventSemaphore",
                                "outs": [],
                                "sync_info": {"on_update": [], "on_wait": [w]},
                            }
                        )
                    si["on_wait"] = waits[-1:]
                out.append(inst)
            blk["instructions"] = out
    return json.dumps(m).encode()


def _build_bass(w_bufs: int = 3, psum_bufs: int = 8, out_bufs: int = 4,
                w_sched=(2, 2, 4) + (8,) * 5 + (4, 4, 2, 2, 2, 1, 1),
                out_sched=(48, 8, 4, 4),
                xa_cols: int = 15, out_eng: str = "gpsimd",
                bias_eng: str = "gpsimd", x0_eng: str = "gpsimd",
                x_eng: str = "sync", final_out_eng: str | None = "sync",
                sync_last_n_outs: int = 1, split_last_tap: int = 4,
                act_bias_last_n: int = 0,
                out_engs=("gpsimd", "gpsimd", "sync", "sync"),
                act_bias_ranges=(),
                bias_engs=((56, 63, "scalar"),),
                split_prefetch: int = 1, bias_in_x: bool = True,
                x_prefetch_all: bool = False, x_fp8: bool = True,
                hoist_head: int = 3, hoist_top: bool = True,
                trim_exit: int = 1, scatter_final: bool = False,
                wb_final: bool = False, wb_outs: bool = True,
                wb_defer_trigger: bool = False,
                reps: int = 1):
    import contextlib

    import concourse.bass as bass
    import concourse.tile as tile
    import concourse.mybir as mybir

    sched = list(w_sched)
    assert sum(sched) == LPC
    osched = list(out_sched)
    assert sum(osched) == LPC
    # w blocks must not straddle out blocks
    obounds = [0]
    for nb in osched:
        obounds.append(obounds[-1] + nb)
    acc = 0
    for nb in sched:
        assert any(a <= acc and acc + nb <= b
                   for a, b in zip(obounds[:-1], obounds[1:]))
        acc += nb

    # x column chunks: [0, xa_cols) then OB-wide chunks to TW
    xbounds = [0, xa_cols]
    while xbounds[-1] < TW:
        xbounds.append(min(xbounds[-1] + OB, TW))

    xdt = mybir.dt.float8e3 if x_fp8 else mybir.dt.bfloat16

    if wb_outs:
        wb_final = True
    nc = bass.Bass(num_swdge_queues=2 if wb_final else 1)
    # bias_in_x: the 128 bias bytes per partition ride as two extra fp8
    # columns at the HEAD of x (cols 0-1), so they move inside x0's single
    # >=512B-descriptor transfer instead of a separate 128B/partition DMA
    # that pays the sub-512B 2x latency penalty (91ns -> +45ns net save)
    xcols = TW + 2 if bias_in_x else TW
    x_d = nc.dram_tensor("x", [IC, xcols, B], xdt, kind="ExternalInput")
    w_d = nc.dram_tensor(
        "w", [IC, LPC, KW, OC], mybir.dt.float8e3, kind="ExternalInput"
    )
    if not bias_in_x:
        b_d = nc.dram_tensor("bias", [OC, LPC], mybir.dt.bfloat16,
                             kind="ExternalInput")
    if scatter_final:
        ix_d = nc.dram_tensor("idx", [128, 8], mybir.dt.int16,
                              kind="ExternalInput")
    o_d = nc.dram_tensor("out", [OC, LPC, B], mybir.dt.bfloat16, kind="ExternalOutput")

    # out DMAs go on their own queue: their compute-dependency waits must not
    # block later weight-block DMAs behind them on SP's in-order sequencer
    oeng = getattr(nc, out_eng)

    with tile.TileContext(nc) as tc:
        with (
            tc.tile_pool(name="const", bufs=1) as constp,
            tc.tile_pool(name="wp", bufs=w_bufs) as wp,
            tc.tile_pool(name="op", bufs=out_bufs) as op,
            tc.tile_pool(name="ps", bufs=psum_bufs, space="PSUM") as pp,
        ):
            # x chunk tiles; chunk 0 lands first so the PE can start early
            xtiles = []  # (start_col, tile)
            nchunks = len(xbounds) - 1
            xoff = 2 if bias_in_x else 0  # bias cols precede x col 0
            for ci in range(nchunks):
                c0, c1 = xbounds[ci], xbounds[ci + 1]
                w_extra = xoff if ci == 0 else 0
                xt = constp.tile([IC, c1 - c0 + w_extra, B], xdt,
                                 name=f"x{ci}", tag=f"x{ci}")
                xtiles.append((c0, xt))
            xdma_done = [False] * nchunks

            def need_x(col):
                ci = next(i for i in range(nchunks)
                          if xbounds[i] <= col < xbounds[i + 1])
                if not xdma_done[ci]:
                    c0, xt = xtiles[ci]
                    d0 = 0 if ci == 0 else c0 + xoff
                    eng = getattr(nc, x0_eng if ci == 0 else x_eng)
                    eng.dma_start(xt[:], x_d[:, d0: d0 + xt.shape[1]])
                    xdma_done[ci] = True
                return ci

            def x_ap(col):
                ci = need_x(col)
                c0, xt = xtiles[ci]
                return xt[:, col - c0 + (xoff if ci == 0 else 0), :]

            need_x(0)
            if bias_in_x:
                # recover the bf16 bias from x0's leading two byte-columns
                bth = (xtiles[0][1][:, 0:2, :]
                       .rearrange("p a b -> p (a b)")
                       .bitcast(mybir.dt.bfloat16))
            else:
                bth = constp.tile([OC, LPC], mybir.dt.bfloat16)
                getattr(nc, bias_eng).dma_start(bth[:], b_d[:])
            fnb = osched[-1]
            ot_final = None
            wb_sem = None
            if wb_final:
                # final out block leaves via a prepared SWDGE kv_writeback:
                # descriptors are generated early on Pool; at the tail a
                # ~60ns trigger_dma (no dispatch/HWDGE/DGE chain) fires the
                # transfer as soon as the last bias-add lands. The whole
                # [OC, fnb, B] block is written as ONE kv "batch" with a
                # (fnb*B)-element contiguous ctx run -> 512B descriptors,
                # dodging the sub-512B 2x penalty.
                def wb_geom(onb):
                    p = 1
                    while p * 2 <= onb and onb % (p * 2) == 0 \
                            and (p * 2) * B <= 2048:
                        p *= 2
                    return onb // p, p  # (kv batch, positions per batch)

                max_bt = max(wb_geom(nb)[0] for nb in osched) if wb_outs else 1
                idxt = constp.tile([128, max_bt], mybir.dt.int32, name="wbidx")
                # memset on Pool: program order guarantees it precedes the
                # (wait-stripped, relocated) preps on the same sequencer
                nc.gpsimd.memset(idxt[:], 0)
                sem_ctx = nc.semaphore("wb_out")
                wb_sem = sem_ctx.__enter__()
                wb_expect = 0

                def wb_view(ol0, onb):
                    bt, p = wb_geom(onb)
                    return (o_d[:, ol0: ol0 + onb, :]
                            .rearrange("(oi oo) (bt li) b -> bt oi oo (li b)",
                                       oo=1, bt=bt))

                fbt, fp = wb_geom(fnb)
                ot_final = op.tile([OC, 1, fbt, fp * B], mybir.dt.bfloat16,
                                   name="otf", tag="otf")
                wb_out_ap = wb_view(LPC - fnb, fnb)
            if scatter_final:
                # final out block goes out via a pre-prepared SWDGE scatter:
                # descriptors are generated early; at the tail only a ~40ns
                # trigger separates the last bias-add from the transfer,
                # replacing the 153+625+650 dispatch+HWDGE+DGE-delay chain.
                # scatter ADDs, so zero the target DRAM region first (early).
                zt = constp.tile([OC, fnb * B], mybir.dt.bfloat16, name="zt")
                nc.vector.memset(zt[:], 0)
                nc.sync.dma_start(o_d[:, LPC - fnb:, :], zt[:])
                idxt = constp.tile([128, 8], mybir.dt.int16, name="idxt")
                nc.sync.dma_start(idxt[:], ix_d[:])
                ot_final = op.tile([OC, 1, fnb * B], mybir.dt.bfloat16,
                                   name="otf", tag="otf")
                import contextlib as _cl
                sem_ctx = nc.semaphore("scat_out")
                scat_sem = sem_ctx.__enter__()
                nc.gpsimd.dma_scatter_add(
                    out_ap=o_d[:, LPC - fnb:, :].opt({0}),
                    in_ap=ot_final[:],
                    idxs_ap=idxt[:],
                    num_idxs=128,
                    num_idxs_reg=128,
                    elem_size=fnb * B,
                    elem_step=LPC * B,
                    prepare_only=True,
                    sem=scat_sem,
                )

            if x_prefetch_all is True:
                # stream order doesn't change when the last w block lands
                # (pool is serial, bytes are bytes), but early x makes every
                # tail-position x-semaphore long-satisfied by drain time
                for ci in range(nchunks):
                    need_x(xbounds[ci])
            # tensor_scalar_add needs an fp32 scalar operand: upcast once on
            # the (otherwise idle) Activation engine, off the critical path
            bt = constp.tile([OC, LPC], mybir.dt.float32)
            nc.scalar.copy(bt[:], bth if bias_in_x else bth[:])

            if reps > 1:  # timing mode: hoist x loads out of the repeat loop
                for ci in range(nchunks):
                    need_x(xbounds[ci])

            blocks = []  # (l0, nb)
            l0 = 0
            for nb in sched:
                blocks.append((l0, nb))
                l0 += nb

            rep_ctx = tc.For_i(0, reps, 1) if reps > 1 else contextlib.nullcontext()
            with rep_ctx:
                bi = 0  # next block to process
                wt = None
                wt_tap = None
                wl0 = wnb = 0
                for ol0, onb in zip(obounds[:-1], osched):
                    is_final = ol0 + onb == LPC
                    wb_block = wb_outs or (wb_final and is_final)
                    if (scatter_final or wb_final) and is_final:
                        ot = ot_final
                        obt, opp = (fbt, fp) if wb_final else (1, onb)
                    elif wb_block:
                        obt, opp = wb_geom(onb)
                        ot = op.tile([OC, 1, obt, opp * B],
                                     mybir.dt.bfloat16,
                                     name=f"ot{onb}", tag=f"ot{onb}")
                    else:
                        ot = op.tile([OC, onb, B], mybir.dt.bfloat16,
                                     name=f"ot{onb}", tag=f"ot{onb}")
                    for j in range(onb):
                        l = ol0 + j
                        if wt is None or l >= wl0 + wnb:
                            wl0, wnb = blocks[bi]
                            bi += 1
                            do_split = (split_last_tap
                                        and blocks[-1][1] == 1)
                            if do_split and bi == len(blocks):
                                # final position: leading taps were prefetched
                                # a block early (below); only the last
                                # split_last_tap taps arrive last, shrinking
                                # the post-arrival critical chain
                                nt = int(split_last_tap)
                                wt = wtf_a
                                wt_tap = wp.tile([IC, 1, nt, OC],
                                                 mybir.dt.float8e3,
                                                 name="wtf_b", tag="wtf_b")
                                nc.sync.dma_start(
                                    wt_tap[:], w_d[:, wl0: wl0 + 1, KW - nt:])
                            else:
                                wt = wp.tile([IC, wnb, KW, OC],
                                             mybir.dt.float8e3,
                                             name=f"wt{wnb}", tag=f"wt{wnb}")
                                wt_tap = None
                                # prefetch x chunks this block touches first
                                need_x(wl0 + wnb - 1 + KW - 1)
                                nc.sync.dma_start(wt[:], w_d[:, wl0: wl0 + wnb])
                            if (isinstance(x_prefetch_all, int)
                                    and x_prefetch_all is not True
                                    and x_prefetch_all > 0
                                    and bi == x_prefetch_all):
                                # deferred full-x prefetch: PE has ramped on
                                # the early blocks; remaining x rides now so
                                # tail x-semaphores are long satisfied
                                for ci in range(nchunks):
                                    need_x(xbounds[ci])
                            if do_split and bi == len(blocks) - split_prefetch:
                                nt = int(split_last_tap)
                                fl0 = blocks[-1][0]
                                wtf_a = wp.tile([IC, 1, KW - nt, OC],
                                                mybir.dt.float8e3,
                                                name="wtf_a", tag="wtf_a")
                                need_x(fl0 + KW - 1)
                                nc.sync.dma_start(
                                    wtf_a[:], w_d[:, fl0: fl0 + 1, : KW - nt])
                        ps = pp.tile([OC, B], mybir.dt.float32)
                        for k in range(KW):
                            if wt_tap is not None and k >= KW - int(split_last_tap):
                                src = wt_tap[:, l - wl0, k - (KW - int(split_last_tap)), :]
                            else:
                                src = wt[:, l - wl0, k, :]
                            nc.tensor.matmul(
                                ps[:],
                                src,
                                x_ap(l + k),
                                start=(k == 0),
                                stop=(k == KW - 1),
                            )
                        if scatter_final and is_final:
                            nc.vector.tensor_scalar_add(
                                ot_final[:, 0, j * B: (j + 1) * B],
                                ps[:], bt[:, l: l + 1]
                            )
                        elif wb_block:
                            jb, jl = divmod(j, opp)
                            tgt = ot[:, 0, jb, jl * B: (jl + 1) * B]
                            weng = next(
                                (e for a, b, e in bias_engs if a <= l < b),
                                "vector")
                            if weng == "scalar":
                                nc.scalar.add(tgt, ps[:], bt[:, l: l + 1])
                            else:
                                getattr(nc, weng).tensor_scalar_add(
                                    tgt, ps[:], bt[:, l: l + 1])
                        elif (beng := next(
                                (e for a, b, e in bias_engs if a <= l < b),
                                "scalar" if (l >= LPC - act_bias_last_n
                                             or any(a <= l < b
                                                    for a, b in act_bias_ranges))
                                else None)) is not None:
                            # tail positions: bias-add off DVE (Activation's
                            # add or Pool's tensor_scalar_add) to dodge DVE's
                            # 216ns/op tail queue
                            if beng == "scalar":
                                nc.scalar.add(ot[:, j, :], ps[:],
                                              bt[:, l: l + 1])
                            else:
                                getattr(nc, beng).tensor_scalar_add(
                                    ot[:, j, :], ps[:], bt[:, l: l + 1])
                        else:
                            nc.vector.tensor_scalar_add(
                                ot[:, j, :], ps[:], bt[:, l: l + 1]
                            )
                    if scatter_final and is_final:
                        nc.gpsimd.trigger_dma(count=None)
                        nc.gpsimd.wait_ge(scat_sem, 1)
                        continue
                    if wb_block:
                        if is_final:
                            # fire blocks 1..3's prepared writebacks now
                            # (one combined trigger, gated by their bias
                            # sems via the preps' deferred deps): their
                            # transfers ride the idle post-stream window
                            # instead of cutting into the weight stream
                            nc.gpsimd.trigger_dma(count=None, queue_num=1)
                        # emitted after the bias-adds so Tile defers the
                        # prep's RAW waits onto the trigger (prep itself is
                        # wait-free and gets hoisted early in IR below)
                        nc.gpsimd.kv_writeback(
                            out_ap=(wb_out_ap if is_final
                                    else wb_view(ol0, onb)),
                            in_ap=ot[:],
                            ctx_idxs_ap=idxt[:, :obt],
                            prepare_only=True,
                            sem=wb_sem,
                            queue_num=1,
                        )
                        wb_expect += 16
                        if is_final:
                            nc.gpsimd.trigger_dma(count=None, queue_num=1)
                        continue
                    oidx = obounds.index(ol0)
                    if out_engs is not None:
                        eng = getattr(nc, out_engs[oidx])
                    else:
                        eng = oeng
                        if final_out_eng is not None and oidx >= len(osched) - sync_last_n_outs:
                            eng = getattr(nc, final_out_eng)
                    eng.dma_start(o_d[:, ol0: ol0 + onb, :], ot[:])

    if scatter_final or wb_final:
        # Tile sinks the prepare next to its trigger at the program tail,
        # putting the ~1us SWDGE descriptor generation on the critical chain
        # (and starving the trigger's no_exec FIFO visit). Move it early: its
        # only wait is the idx tile (memset/DMA, ~2us); parking Pool's
        # sequencer on that is harmless since the next Pool work (out
        # dispatches) is much later.
        prep_ty = ("InstDMAScatterAddAnt" if scatter_final
                   else "InstKVWritebackAnt")
        body = nc.m.functions[0].blocks[1].instructions
        preps = [inst for inst in body if type(inst).__name__ == prep_ty]
        prep = preps[0]
        if wb_final:
            # The prep's data read happens at trigger time, but the emitted
            # sync waits (on the bias-adds that fill its source tile) sit on
            # the prep and would park Pool's sequencer until the tail. MOVE
            # each prep's waits onto its trigger — the trigger is the actual
            # read point, so the data dependency stays sound while the prep
            # (descriptor gen only) runs early. The prep's remaining dep, the
            # idxt memset, precedes it in Pool program order.
            for _p in preps:
                _si = getattr(_p, "sync_info", None)
                if _si is None or not getattr(_si, "on_wait", None):
                    continue
                _trig = None
                _seen = False
                for _inst in body:
                    if _inst is _p:
                        _seen = True
                    elif _seen and type(_inst).__name__ == "InstTriggerDma":
                        _trig = _inst
                        break
                assert _trig is not None, "prep without trigger"
                _tsi = _trig.sync_info
                _have = {(w.id, w.wait_mode): w
                         for w in (_tsi.on_wait or [])}
                for w in _si.on_wait:
                    k = (w.id, w.wait_mode)
                    if k in _have:
                        _have[k].wait_value = max(
                            _have[k].wait_value or 0, w.wait_value or 0)
                    else:
                        _tsi.on_wait.append(w)
                        _have[k] = w
                del _si.on_wait[:]
        if True:
            kept = [inst for inst in body
                    if type(inst).__name__ != prep_ty]
            del body[:]
            body.extend(kept)
        import concourse.mybir as _mb
        n_pool = 0
        ins_at = 0
        for i, inst in enumerate(body):
            if (inst.engine == _mb.EngineType.Pool
                    and type(inst).__name__ == "InstDMACopy"):
                n_pool += 1
                if n_pool == 2:  # after x0 and bias dispatches
                    ins_at = i + 1
                    break
        if wb_final:
            # the preps' descriptor-gen READS the idxt tile at gen time:
            # they must land after its Pool memset (and after the library
            # reload), or the Q7 reads garbage ctx indices and the OOB
            # guard silently skips every write
            for i in range(ins_at, len(body)):
                inst = body[i]
                if (inst.engine == _mb.EngineType.Pool
                        and type(inst).__name__ == "InstMemset"):
                    ins_at = i + 1
                    break
        for k, _p in enumerate(preps):
            body.insert(ins_at + k, _p)
    if wb_final:
        # The framework's exit flush-drain waits every DMA-queue sem to its
        # expected final value, but the prepared writeback posts its 16
        # completion bumps to the custom wb_out sem instead of its SWDGE
        # queue's builtin sem. Clamp each drain wait to what the program
        # actually posts (drop if nothing does) and wait the wb sem
        # explicitly so the drain still covers the writeback's landing.
        posted: dict[int, int] = {}
        wb_id = None
        for blk in nc.m.functions[0].blocks:
            for inst in blk.instructions:
                si = getattr(inst, "sync_info", None)
                for upd in (getattr(si, "on_update", None) or []):
                    if str(getattr(upd, "update_mode", "")) in (
                            "sem-inc", "sem-add-imm"):
                        posted[upd.id] = posted.get(upd.id, 0) + (
                            upd.update_value or 0)
                        if (upd.ant_name or "").startswith("wb_out"):
                            wb_id = upd.id
        import concourse.mybir as _mb2
        for inst in nc.m.functions[0].blocks[-1].instructions:
            si = getattr(inst, "sync_info", None)
            waits = (getattr(si, "on_wait", None) or [])
            if not waits or type(inst).__name__ != "InstDrain":
                continue
            new_waits = []
            patched = False
            for w in waits:
                nm = w.ant_name or ""
                if (nm.startswith("DMASW") or nm.startswith("DMAHW")):
                    have = posted.get(w.id, 0)
                    if have <= 0:
                        patched = True
                        continue  # nothing posts: unsatisfiable, drop
                    if have < (w.wait_value or 0):
                        w.wait_value = have
                        patched = True
                new_waits.append(w)
            if patched and wb_id is not None:
                new_waits.append(_mb2.SyncWait(
                    sync_type="semaphore", id=wb_id, ant_name="wb_out",
                    wait_mode="sem-ge-imm",
                    wait_value=posted.get(wb_id, 16), wait_reg=None))
            if patched:
                del si.on_wait[:]
                si.on_wait.extend(new_waits)
    if wb_final:
        # Move the exit flush-drain's wait list onto Pool's gather-waiter
        # EventSemaphore: every instruction and the barrier protocol stay
        # intact, but the waits land on the FINAL Pool ISA: the barrier
        # gather/release and Pool's drain all run early (none depend on
        # the writeback landing - the SWDGE ring was consumed at trigger
        # time), and only the true last instruction observes the wb
        # semaphore, trimming the post-drain chain to one instruction.
        _ex = nc.m.functions[0].blocks[-1].instructions
        _flush = _ex[0]
        _fsi = getattr(_flush, "sync_info", None)
        if (type(_flush).__name__ == "InstDrain" and _fsi is not None
                and len(_fsi.on_wait or []) > 4):
            _tgt = next(
                (i for i in reversed(_ex)
                 if type(i).__name__ == "InstISA"
                 and str(i.engine).endswith("Pool")),
                None)
            if _tgt is not None:
                _tsi = _tgt.sync_info
                if _tsi is None:
                    import concourse.mybir as _mbx
                    _tgt.sync_info = _mbx.SyncInfo(on_wait=[], on_update=[])
                    _tsi = _tgt.sync_info
                _moved = list(_fsi.on_wait)
                # keep the latest-firing wait (wb_out) last on the
                # instruction itself; earlier ones become cheap pre-waits
                _moved.sort(key=lambda w: (w.ant_name or "") == "wb_out")
                _keep = list(_tsi.on_wait)
                del _tsi.on_wait[:]
                _tsi.on_wait.extend(_keep + _moved)
                del _fsi.on_wait[:]
    if hoist_head > 0:
        _hoist_head_dmas_ir(nc, n=hoist_head, top=hoist_top)
    if trim_exit:
        _trim_exit_barrier_ir(nc, deep=(trim_exit == 2))
    if wb_final or scatter_final:
        # kv_writeback's Q7 ucode lives in the reloadable 'attn' library,
        # not the boot default. Bacc.compile runs the insert_library_loads
        # pass to place MODIFY_POOL_CONFIG loads before instructions that
        # need a non-resident library (and back-switches for standard-lib
        # ops); raw Bass never does, so run the same rust pass here.
        from concourse.library_config import all_libraries, standard
        import bass_rust as _br
        _mask: dict = {}
        for _lib in all_libraries:
            for _it in _lib.instructions:
                _mask[_it] = _mask.get(_it, 0) | (1 << _lib.index)
        _br.insert_library_loads(nc, _mask, len(all_libraries),
                                 standard.index)
        # extended insts (kv_writeback / trigger_dma / IncSwdgeSem) carry
        # their ISA encodings in .instr — raw Bass never populates them
        # (Bacc.compile does); without this the NEFF codegen fails with
        # "ISA wrong length"
        from concourse.library_overlay import lower_extended_insts
        lower_extended_insts(nc)
    fixed = _split_waits(bass.Bass.to_json_bytes(nc))
    nc.to_json_bytes = lambda: fixed  # type: ignore[method-assign]
    return nc


def _prepare_inputs(x, weight, bias, x_fp8=True, bias_in_x=True):
    x = np.asarray(x, dtype=np.float32)
    weight = np.asarray(weight, dtype=np.float32)
    bias = np.asarray(bias, dtype=np.float32)

    # x: [b, i, t] -> bf16/fp8, pad t to TPAD, transpose -> [i, t, b]
    xdt = _F8 if x_fp8 else _BF16
    xpad = np.zeros((B, IC, TPAD), dtype=xdt)
    xpad[:, :, :LIN] = x.astype(xdt)
    xt = xpad.transpose(1, 2, 0)  # [i, t, b] view

    # weight: [l, o, i, k] -> fp8 e3m4, pad l, transpose -> [i, l, k, o]
    wpad = np.zeros((NCORES * LPC, OC, IC, KW), dtype=_F8)
    wpad[:LOUT] = weight.astype(_F8)
    wt = wpad.transpose(2, 0, 3, 1)  # [i, l, k, o] view

    bpad = np.zeros((OC, NCORES * LPC), dtype=_BF16)
    bpad[:, :LOUT] = bias.astype(_BF16)

    in_maps = []
    for c in range(NCORES):
        l0 = c * LPC
        if bias_in_x:
            # bias [OC, LPC] bf16 -> 128 raw bytes per partition -> two
            # leading fp8 byte-columns of x (matches the on-chip bitcast:
            # free-dim-contiguous little-endian bf16 pairs)
            bb = (np.ascontiguousarray(bpad[:, l0: l0 + LPC])
                  .view(np.uint8)          # [OC, 2*LPC]
                  .reshape(IC, 2, B)
                  .view(xdt if x_fp8 else np.uint8))
            if not x_fp8:
                raise NotImplementedError("bias_in_x requires x_fp8")
            xc = np.concatenate(
                [bb, np.ascontiguousarray(xt[:, l0: l0 + TW, :])], axis=1)
            in_maps.append(
                {
                    "x": np.ascontiguousarray(xc),
                    "w": np.ascontiguousarray(wt[:, l0: l0 + LPC]),
                }
            )
        else:
            in_maps.append(
                {
                    "x": np.ascontiguousarray(xt[:, l0: l0 + TW, :]),
                    "w": np.ascontiguousarray(wt[:, l0: l0 + LPC]),
                    "bias": np.ascontiguousarray(bpad[:, l0: l0 + LPC]),
                }
            )
    return in_maps


def _assemble(results):
    full = np.stack([results[c]["out"] for c in range(NCORES)], axis=0)
    # [c, o, l_loc, b] (bf16) -> fp32 [b, o, c*LPC + l_loc] -> crop to LOUT
    out = (
        full.astype(np.float32)
        .transpose(3, 1, 0, 2)
        .reshape(B, OC, NCORES * LPC)[:, :, :LOUT]
    )
    return np.ascontiguousarray(out)


def kernel(x, weight, bias):
    global LAST_RESULTS
    import time

    from concourse.bass_utils import run_bass_kernel_spmd

    # attempt order: fast prepared-writeback build (29011ns), retried once
    # after a pause (a transiently wedged device recovers in ~20-60s), then
    # the plain-DMA fallback build (30491ns) which uses no extended
    # instructions at all
    attempts = [
        ("wb", dict(), 0),
        ("wb", dict(), 25),
        ("nowb", dict(wb_outs=False, bias_in_x=False, trim_exit=1,
                      out_bufs=3), 20),
    ]
    last_exc = None
    for key, build_kw, delay in attempts:
        if delay:
            time.sleep(delay)
        try:
            if _CACHE.get(key) is None:
                _CACHE[key] = _build_bass(**build_kw)
            nc = _CACHE[key]
            in_maps = _prepare_inputs(
                x, weight, bias,
                bias_in_x=build_kw.get("bias_in_x", True))
            res = run_bass_kernel_spmd(nc, in_maps,
                                       core_ids=list(range(NCORES)))
            LAST_RESULTS = res
            _CACHE["nc"] = nc  # for test.py's TimelineSim fallback
            return _assemble(res.results)
        except Exception as e:  # noqa: BLE001 - device/compile faults
            last_exc = e
    raise last_exc

